# revision 1
# baseline (speedup 1.0000x reference)
"""Multi-head attention (B=2, S=2048, D=1024, H=16, causal, interleaved RoPE)
on 8 Trainium2 NeuronCores.

Sharding: tensor-parallel over heads — 2 heads (128 channels) per core.
Each core computes its Q/K/V projections, RoPE, causal attention, and a
row-parallel partial of the output projection; the host sums the partials.

Matmuls run in bf16 with fp32 PSUM accumulation (RoPE, softmax denominators
and all epilogues in fp32).

Layout:
  * Q/K projection weights are fed with output channels permuted so each
    head's dims are [evens(32), odds(32)] -> the RoPE pair-swap becomes a
    32-partition-block swap done with SBUF->SBUF DMAs; RoPE itself is three
    fp32 DVE multiplies/adds fused into the projection epilogue.
  * Attention uses the S^T layout: scores psum [k(128part), q(512)] via
    matmul(lhsT=K^T, rhs=Q^T), with the two heads issued back-to-back on
    disjoint PE row groups (partitions 0-63 / 64-127) so their weight loads
    overlap the other head's matmul. Softmax needs no max subtraction
    (scores are O(10)); exp on ACT writes bf16 P^T; causal masking is a
    multiply by a 0/1 slice of a [128,1024] band tile; PV via
    matmul(lhsT=V_aug, rhs=P^T) where V_aug carries a ones column so the
    denominator drops out as psum row 64; 1/denom (fast-approx reciprocal)
    is broadcast across partitions with a K=1 matmul and applied on the
    PV psum -> y^T copy.
  * x^T (contraction over D needs D on partitions) is produced on-device:
    cast to bf16 during the DMA (SWDGE), then PE transposes.
"""

import numpy as np
import ml_dtypes

import concourse.bacc as bacc
import concourse.mybir as mybir
import concourse.tile as tile
from concourse.bass_utils import run_bass_kernel_spmd
from concourse.masks import make_identity

P = 128
B, S, D = 2, 2048, 1024
H, DH = 16, 64
NROWS = B * S            # 4096 flattened rows
CH = 128                 # channels per core (2 heads)
RB = 512                 # row block for projections / q tiles
NRB = NROWS // RB        # 8
DSUB = D // P            # 8 contraction subtiles
KSUB = NROWS // P        # 32 k subtiles (128 rows each)
QT_PER_B = S // RB       # 4 q tiles per batch
ROPE_BASE = 10000.0

f32 = mybir.dt.float32
f32r = mybir.dt.float32r
bf16 = mybir.dt.bfloat16
import os as _os
USE_BF16 = _os.environ.get("KDT", "f32r") == "bf16"
MMDT = bf16 if USE_BF16 else f32r
MMNP = ml_dtypes.bfloat16 if USE_BF16 else np.float32

_CACHE = {}


def _build():
    nc = bacc.Bacc("TRN2", target_bir_lowering=False)

    x_ext = nc.declare_dram_parameter("x", [NROWS, D], f32 if USE_BF16 else f32r, isOutput=False)
    wqT_ext = nc.declare_dram_parameter("wqT", [D, CH], MMDT, isOutput=False)
    wkT_ext = nc.declare_dram_parameter("wkT", [D, CH], MMDT, isOutput=False)
    wvT_ext = nc.declare_dram_parameter("wvT", [D, CH], MMDT, isOutput=False)
    woT_ext = nc.declare_dram_parameter("woT", [CH, D], MMDT, isOutput=False)
    bq_ext = nc.declare_dram_parameter("bq", [CH, 1], f32, isOutput=False)
    bk_ext = nc.declare_dram_parameter("bk", [CH, 1], f32, isOutput=False)
    bv_ext = nc.declare_dram_parameter("bv", [CH, 1], f32, isOutput=False)
    cc_ext = nc.declare_dram_parameter("cc", [P, NROWS], f32, isOutput=False)
    ss_ext = nc.declare_dram_parameter("ss", [P, NROWS], f32, isOutput=False)
    mask_ext = nc.declare_dram_parameter("mask", [P, 1024], MMDT, isOutput=False)
    out_ext = nc.declare_dram_parameter("out", [NROWS, D], f32, isOutput=True)

    with tile.TileContext(nc) as tc:
        with (
            tc.tile_pool(name="const", bufs=1) as cpool,
            tc.tile_pool(name="big", bufs=1) as big,
            tc.tile_pool(name="work", bufs=2) as work,
            tc.tile_pool(name="small", bufs=3) as small,
            tc.tile_pool(name="ptpool", bufs=6) as ptpool,
            tc.tile_pool(name="psumA", bufs=2, space="PSUM") as psumA,
            tc.tile_pool(name="psumB", bufs=1, space="PSUM") as psumB,
        ):
            # ---- constants ----
            ident_f = cpool.tile([P, P], f32, tag="identf")
            make_identity(nc, ident_f[:])
            ident = cpool.tile([P, P], MMDT, tag="ident")
            nc.vector.tensor_copy(ident[:], ident_f[:])

            ones_f = cpool.tile([P, 64], f32, tag="onesf")
            nc.vector.memset(ones_f[:], 1.0)
            ones_b = cpool.tile([P, 64], MMDT, tag="onesb")
            nc.vector.tensor_copy(ones_b[:], ones_f[:])
            ones_r = cpool.tile([P, 64], f32r, tag="onesr")
            nc.vector.tensor_copy(ones_r[:], ones_f[:])

            wq_sb = cpool.tile([P, DSUB, CH], MMDT, tag="wq")
            wk_sb = cpool.tile([P, DSUB, CH], MMDT, tag="wk")
            wv_sb = cpool.tile([P, DSUB, CH], MMDT, tag="wv")
            for d in range(DSUB):
                nc.sync.dma_start(wq_sb[:, d], wqT_ext[d * P:(d + 1) * P, :])
                nc.sync.dma_start(wk_sb[:, d], wkT_ext[d * P:(d + 1) * P, :])
                nc.sync.dma_start(wv_sb[:, d], wvT_ext[d * P:(d + 1) * P, :])
            wo_sb = cpool.tile([CH, D], MMDT, tag="wo")
            nc.sync.dma_start(wo_sb[:, 0:512], woT_ext[:, 0:512])
            nc.sync.dma_start(wo_sb[:, 512:1024], woT_ext[:, 512:1024])
            bq_sb = cpool.tile([CH, 1], f32, tag="bq")
            nc.sync.dma_start(bq_sb[:], bq_ext[:])
            bk_sb = cpool.tile([CH, 1], f32, tag="bk")
            nc.sync.dma_start(bk_sb[:], bk_ext[:])
            bv_sb = cpool.tile([CH, 1], f32, tag="bv")
            nc.sync.dma_start(bv_sb[:], bv_ext[:])
            mask_sb = cpool.tile([P, 1024], MMDT, tag="mask")
            nc.sync.dma_start(mask_sb[:], mask_ext[:])

            # ---- persistent activation tiles ----
            qT = big.tile([P, NROWS], MMDT, tag="qT")     # roped Q^T (pre-scaled 1/8)
            kT = big.tile([P, NROWS], MMDT, tag="kT")     # roped K^T
            yT = big.tile([P, NROWS], MMDT, tag="yT")     # attention out ^T
            v_sb = big.tile([P, KSUB, 130], MMDT, tag="v")  # V natural + ones cols

            # ones columns of v (cols 64 and 129 of each k-subtile)
            nc.vector.tensor_copy(
                v_sb[:, :, 64:130:65].rearrange("p a b -> p (a b)"),
                ones_b[:, 0:2 * KSUB])

            # ====== phase A: x^T, projections, RoPE (fused per row block) ======
            proj_list = [
                ("q", wq_sb, bq_sb, 0.125, qT),
                ("k", wk_sb, bk_sb, 1.0, kT),
                ("v", wv_sb, bv_sb, 1.0, None),
            ]
            for rt in range(NRB):               # 8 blocks of 512 rows
                sl = slice(rt * RB, (rt + 1) * RB)
                xT = work.tile([P, DSUB, RB], MMDT, tag="xT")
                for rc in range(RB // P):       # 4 chunks of 128 rows
                    r0 = rt * RB + rc * P
                    xab = work.tile([P, D], MMDT, tag="xab")
                    if USE_BF16:
                        # cast fp32 -> bf16 during the DMA (SWDGE)
                        nc.gpsimd.dma_start(xab[:], x_ext[r0:r0 + P, :])
                    else:
                        nc.sync.dma_start(xab[:], x_ext[r0:r0 + P, :])
                    for half in range(2):
                        tp = psumA.tile([P, 512], MMDT, tag="tp")
                        for j in range(4):
                            d = half * 4 + j
                            nc.tensor.transpose(
                                tp[:, j * P:(j + 1) * P],
                                xab[:, d * P:(d + 1) * P], ident[:])
                        nc.vector.tensor_copy(
                            xT[:, half * 4:(half + 1) * 4, rc * P:(rc + 1) * P],
                            tp[:].rearrange("p (j c) -> p j c", j=4))

                ccc = small.tile([P, RB], f32, tag="ccc")
                nc.sync.dma_start(ccc[:], cc_ext[:, sl])
                sss = small.tile([P, RB], f32, tag="sss")
                nc.sync.dma_start(sss[:], ss_ext[:, sl])

                for name, w_sb, b_sb, scale, dstT in proj_list:
                    pp = psumA.tile([P, RB], f32, tag="proj")
                    for d in range(DSUB):
                        nc.tensor.matmul(pp[:], w_sb[:, d], xT[:, d],
                                         start=(d == 0), stop=(d == DSUB - 1))
                    if name != "v":
                        praw = work.tile([P, RB], f32, tag="praw")
                        nc.scalar.activation(
                            praw[:], pp[:],
                            mybir.ActivationFunctionType.Identity,
                            bias=b_sb[:, 0:1], scale=scale)
                        # RoPE: dst = praw*cc + swap32(praw)*ss  (fp32, ->bf16)
                        xsw = work.tile([P, RB], f32, tag="xsw")
                        for hh in range(2):
                            b0 = hh * 64
                            nc.sync.dma_start(xsw[b0:b0 + 32, :],
                                              praw[b0 + 32:b0 + 64, :])
                            nc.sync.dma_start(xsw[b0 + 32:b0 + 64, :],
                                              praw[b0:b0 + 32, :])
                        t1 = small.tile([P, RB], f32, tag="ropet1")
                        nc.vector.tensor_mul(t1[:], praw[:], ccc[:])
                        t2 = small.tile([P, RB], f32, tag="ropet2")
                        nc.vector.tensor_mul(t2[:], xsw[:], sss[:])
                        nc.vector.tensor_add(dstT[:, sl], t1[:], t2[:])
                    else:
                        # V^T chunk with bias, then PE-transpose to natural V
                        vr = work.tile([P, RB], MMDT, tag="vraw")
                        nc.scalar.activation(
                            vr[:], pp[:],
                            mybir.ActivationFunctionType.Identity,
                            bias=b_sb[:, 0:1], scale=1.0)
                        tpv = psumA.tile([P, 512], MMDT, tag="tp")
                        for rc2 in range(4):
                            nc.tensor.transpose(
                                tpv[:, rc2 * P:(rc2 + 1) * P],
                                vr[:, rc2 * P:(rc2 + 1) * P], ident[:])
                        # scatter: head0 chans -> cols 0:64, head1 -> cols 65:129
                        tpv_v = tpv[:].rearrange("p (k h c) -> p k h c", k=4, h=2)
                        vdst = (v_sb[:, rt * 4:(rt + 1) * 4, 0:130]
                                .rearrange("p k (h c) -> p k h c", h=2))
                        for hh in range(2):
                            nc.vector.tensor_copy(vdst[:, :, hh, 0:64],
                                                  tpv_v[:, :, hh, :])

            # ================= phase C: attention =================
            for b in range(B):
                for qt in range(QT_PER_B):
                    qcols = slice(b * S + qt * RB, b * S + (qt + 1) * RB)
                    nks = qt * 4 + 4
                    pvs = []
                    for h in range(2):
                        pv_t = psumB.tile([65, RB], f32, tag=f"pv{h}",
                                          name=f"pv{h}_{b}_{qt}")
                        pvs.append(pv_t)
                    for ks in range(nks):
                        kcols = slice(b * S + ks * P, b * S + (ks + 1) * P)
                        ksg = b * (S // P) + ks
                        m = ks - qt * 4
                        # diagonal blocks: only q columns j >= m*128 are valid
                        j0 = m * P if m >= 1 else 0
                        qv = slice(b * S + qt * RB + j0, b * S + (qt + 1) * RB)
                        pts = []
                        for h in range(2):
                            hsl = slice(h * 64, (h + 1) * 64)
                            st = psumA.tile([P, RB], f32, tag="st")
                            nc.tensor.matmul(st[:, j0:], kT[hsl, kcols],
                                             qT[hsl, qv],
                                             start=True, stop=True)
                            pt = ptpool.tile([P, RB], MMDT, tag="pt")
                            nc.scalar.activation(pt[:, j0:], st[:, j0:],
                                                 mybir.ActivationFunctionType.Exp)
                            if m >= 0:
                                off = 512 - m * P
                                nc.vector.tensor_mul(pt[:, j0:], pt[:, j0:],
                                                     mask_sb[:, off + j0:off + RB])
                            pts.append(pt)
                        for h in range(2):
                            nc.tensor.matmul(
                                pvs[h][:, j0:], v_sb[:, ksg, h * 65:(h + 1) * 65],
                                pts[h][:, j0:],
                                start=(ks == 0), stop=(ks == nks - 1))
                    for h in range(2):
                        pv = pvs[h]
                        rcp_f = small.tile([65, RB], f32, tag="rcpf")
                        with nc.allow_low_precision(reason="fp32 recip of fp32"):
                            nc.vector.reciprocal(rcp_f[64:65, :], pv[64:65, :])
                        rcp_r = small.tile([65, RB], f32r, tag="rcpr")
                        nc.vector.tensor_copy(rcp_r[64:65, :], rcp_f[64:65, :])
                        rep = psumA.tile([64, RB], f32, tag="tp")
                        nc.tensor.matmul(rep[:], ones_r[64:65, 0:64],
                                         rcp_r[64:65, :], start=True, stop=True)
                        rep_sb = small.tile([64, RB], f32, tag="repsb")
                        nc.scalar.copy(rep_sb[:], rep[:])
                        if h == 0:
                            nc.vector.tensor_mul(yT[0:64, qcols], pv[0:64, :],
                                                 rep_sb[:])
                        else:
                            t64 = small.tile([64, RB], MMDT, tag="t64")
                            nc.vector.tensor_mul(t64[:], pv[0:64, :], rep_sb[:])
                            nc.sync.dma_start(yT[64:128, qcols], t64[:])

            # ================= phase D: output projection =================
            for rt in range(KSUB):              # 32 tiles of 128 rows
                for ec in range(2):
                    op = psumA.tile([P, 512], f32, tag="proj")
                    nc.tensor.matmul(op[:], yT[:, rt * P:(rt + 1) * P],
                                     wo_sb[:, ec * 512:(ec + 1) * 512],
                                     start=True, stop=True)
                    ob = small.tile([P, 512], f32, tag="ob")
                    nc.vector.tensor_copy(ob[:], op[:])
                    nc.sync.dma_start(
                        out_ext[rt * P:(rt + 1) * P, ec * 512:(ec + 1) * 512],
                        ob[:])

    nc.finalize()
    return nc


def _host_inputs():
    t = np.arange(32, dtype=np.float64)
    inv_freq = 1.0 / (ROPE_BASE ** (2.0 * t / DH))
    pos = np.arange(S, dtype=np.float64)
    ang = pos[None, :] * inv_freq[:, None]          # [32, S]
    cos32 = np.cos(ang).astype(np.float32)
    sin32 = np.sin(ang).astype(np.float32)
    cos32 = np.tile(cos32, (1, B))                  # [32, 4096]
    sin32 = np.tile(sin32, (1, B))
    cc = np.tile(cos32, (4, 1))                     # [128, 4096]
    ss = np.concatenate([-sin32, sin32, -sin32, sin32], axis=0)  # [128, 4096]

    ii = np.arange(P)[:, None]
    jj = np.arange(1024)[None, :]
    mask = (jj >= ii + 512).astype(np.float32)      # [128, 1024]

    perm64 = np.concatenate([np.arange(0, 64, 2), np.arange(1, 64, 2)])
    return cc, ss, mask, perm64


def _in_maps(x, Wq, bq, Wk, bk, Wv, bv, Wo):
    cc, ss, mask, perm64 = _host_inputs()
    x2 = np.ascontiguousarray(x.reshape(NROWS, D))
    perm128 = np.concatenate([perm64, perm64 + 64])
    maps = []
    for c in range(8):
        sl = slice(c * CH, (c + 1) * CH)
        maps.append({
            "x": x2,
            "wqT": np.ascontiguousarray(Wq[sl][perm128].T).astype(MMNP),
            "wkT": np.ascontiguousarray(Wk[sl][perm128].T).astype(MMNP),
            "wvT": np.ascontiguousarray(Wv[sl].T).astype(MMNP),
            "woT": np.ascontiguousarray(Wo[:, sl].T).astype(MMNP),
            "bq": (bq[sl][perm128] * 0.125).reshape(CH, 1).copy(),
            "bk": bk[sl][perm128].reshape(CH, 1).copy(),
            "bv": bv[sl].reshape(CH, 1).copy(),
            "cc": cc, "ss": ss, "mask": mask.astype(MMNP),
        })
    return maps


def kernel(x, Wq, bq, Wk, bk, Wv, bv, Wo, bo):
    x = np.asarray(x, dtype=np.float32)
    Wq = np.asarray(Wq, dtype=np.float32)
    Wk = np.asarray(Wk, dtype=np.float32)
    Wv = np.asarray(Wv, dtype=np.float32)
    Wo = np.asarray(Wo, dtype=np.float32)
    bq = np.asarray(bq, dtype=np.float32)
    bk = np.asarray(bk, dtype=np.float32)
    bv = np.asarray(bv, dtype=np.float32)
    bo = np.asarray(bo, dtype=np.float32)

    if "nc" not in _CACHE:
        _CACHE["nc"] = _build()
    nc = _CACHE["nc"]

    res = run_bass_kernel_spmd(nc, _in_maps(x, Wq, bq, Wk, bk, Wv, bv, Wo),
                               core_ids=list(range(8)))
    out = np.zeros((NROWS, D), dtype=np.float32)
    for r in res.results:
        out += r["out"]
    out += bo[None, :]
    return out.reshape(B, S, D)



# revision 20
# speedup vs baseline: 1.0672x; 1.0672x over previous
"""Multi-head attention (B=2, S=2048, D=1024, H=16, causal, interleaved RoPE)
on 8 Trainium2 NeuronCores.

Sharding: tensor-parallel over heads - 2 heads (128 channels) per core.
Each core computes its Q/K/V projections, RoPE, causal attention, and a
row-parallel partial of the output projection; the host sums the bf16
partials in fp32.

All matmuls in bf16 with fp32 PSUM accumulation. Key layout choices:
  * x^T is pre-transposed and cast to bf16 on the host (block-major layout)
    so no on-device transposes are needed for the projections.
  * Q/K projection weights are fed with output channels permuted so each
    head's dims are [evens(32), odds(32)] -> the RoPE pair-swap becomes a
    32-partition-block swap done with SBUF->SBUF DMAs; RoPE itself is three
    bf16 DVE ops on fused [128,2,512] (q|k) tiles.
  * Attention uses the S^T layout: scores psum [k(128part), q(512)] via
    matmul(lhsT=K^T, rhs=Q^T), the two heads concurrent on disjoint PE row
    groups (partitions 0-63 / 64-127) writing the two banks of one
    [128,1024] psum tile; ONE exp over both heads; causal masking is a
    gpsimd multiply of the [128,128] diagonal strip by a triangular const;
    PV via matmul(lhsT=V_aug, rhs=P^T) where V_aug carries a ones column so
    the denominator drops out as psum row 64; 1/denom via fast-approx
    reciprocal straight off the psum row, broadcast across partitions with
    a gpsimd partition_broadcast (PE/ACT untouched).
  * Emission interleaves projection blocks, attention q-tiles, and output-
    projection chunks so the PE stays dense (HAM-warm) and exp overlaps
    projections.
"""

import os
import numpy as np
import ml_dtypes

import concourse.bacc as bacc
import concourse.mybir as mybir
import concourse.tile as tile
from concourse.bass_utils import run_bass_kernel_spmd
from concourse.masks import make_identity

P = 128
B, S, D = 2, 2048, 1024
H, DH = 16, 64
NROWS = B * S            # 4096 flattened rows
CH = 128                 # channels per core (2 heads)
RB = 512                 # row block for projections / q tiles
NRB = NROWS // RB        # 8
DSUB = D // P            # 8 contraction subtiles
KSUB = NROWS // P        # 32 k subtiles (128 rows each)
QT_PER_B = S // RB       # 4 q tiles per batch
ROPE_BASE = 10000.0

f32 = mybir.dt.float32
bf16 = mybir.dt.bfloat16
nbf16 = ml_dtypes.bfloat16

_CACHE = {}


def _build():
    nc = bacc.Bacc("TRN2", target_bir_lowering=False)

    xT_ext = nc.declare_dram_parameter("xT", [P, NRB * DSUB * RB], bf16,
                                       isOutput=False)
    wqT_ext = nc.declare_dram_parameter("wqT", [D, CH], bf16, isOutput=False)
    wkT_ext = nc.declare_dram_parameter("wkT", [D, CH], bf16, isOutput=False)
    wvT_ext = nc.declare_dram_parameter("wvT", [D, CH], bf16, isOutput=False)
    woT_ext = nc.declare_dram_parameter("woT", [CH, D], bf16, isOutput=False)
    bq_ext = nc.declare_dram_parameter("bq", [CH, 1], f32, isOutput=False)
    bk_ext = nc.declare_dram_parameter("bk", [CH, 1], f32, isOutput=False)
    bv_ext = nc.declare_dram_parameter("bv", [CH, 1], f32, isOutput=False)
    cc_ext = nc.declare_dram_parameter("cc2", [P, 2 * S], bf16, isOutput=False)
    ss_ext = nc.declare_dram_parameter("ss2", [P, 2 * S], bf16, isOutput=False)
    tri_ext = nc.declare_dram_parameter("tri", [P, 2 * P], bf16, isOutput=False)
    out_ext = nc.declare_dram_parameter("out", [NROWS, D], bf16, isOutput=True)
    DBG = bool(os.environ.get("KDBG"))
    if DBG:
        dbg_dn = nc.declare_dram_parameter("dbg_dn", [1, 1024], f32,
                                           isOutput=True)
        dbg_dcp = nc.declare_dram_parameter("dbg_dcp", [1, 1024], f32,
                                            isOutput=True)
        dbg_rep = nc.declare_dram_parameter("dbg_rep", [64, 1024], f32,
                                            isOutput=True)
        dbg_qk = nc.declare_dram_parameter("dbg_qk", [P, 1024], bf16,
                                           isOutput=True)
        dbg_v = nc.declare_dram_parameter("dbg_v", [P, 1024], bf16,
                                          isOutput=True)
        dbg_yt = nc.declare_dram_parameter("dbg_yt", [P, 512], bf16,
                                           isOutput=True)

    with tile.TileContext(nc) as tc:
        with (
            tc.tile_pool(name="const", bufs=1) as cpool,
            tc.tile_pool(name="xpool", bufs=NRB) as xpool,
            tc.tile_pool(name="big", bufs=1) as big,
            tc.tile_pool(name="work", bufs=2) as work,
            tc.tile_pool(name="ptp", bufs=4) as ptp,
            tc.tile_pool(name="small", bufs=2) as small,
            tc.tile_pool(name="obp", bufs=3) as obp,
            tc.tile_pool(name="psum", bufs=2, space="PSUM") as psum,
            tc.tile_pool(name="psumpv", bufs=4, space="PSUM") as psumpv,
        ):
            # ---- input DMAs (xT blocks first so block 0 lands early) ----
            xTb = []
            for rt in range(NRB):
                xt = xpool.tile([P, DSUB, RB], bf16, tag="xT",
                                name=f"xT{rt}")
                nc.sync.dma_start(
                    xt[:].rearrange("p d c -> p (d c)"),
                    xT_ext[:, rt * DSUB * RB:(rt + 1) * DSUB * RB])
                xTb.append(xt)

            wq_sb = cpool.tile([P, DSUB, CH], bf16, tag="wq")
            wk_sb = cpool.tile([P, DSUB, CH], bf16, tag="wk")
            wv_sb = cpool.tile([P, DSUB, CH], bf16, tag="wv")
            for d in range(DSUB):
                nc.sync.dma_start(wq_sb[:, d], wqT_ext[d * P:(d + 1) * P, :])
                nc.sync.dma_start(wk_sb[:, d], wkT_ext[d * P:(d + 1) * P, :])
                nc.sync.dma_start(wv_sb[:, d], wvT_ext[d * P:(d + 1) * P, :])
            bq_sb = cpool.tile([CH, 1], f32, tag="bq")
            nc.sync.dma_start(bq_sb[:], bq_ext[:])
            bk_sb = cpool.tile([CH, 1], f32, tag="bk")
            nc.sync.dma_start(bk_sb[:], bk_ext[:])
            bv_sb = cpool.tile([CH, 1], f32, tag="bv")
            nc.sync.dma_start(bv_sb[:], bv_ext[:])
            cc_sb = cpool.tile([P, 2, S], bf16, tag="cc")
            nc.sync.dma_start(cc_sb[:].rearrange("p a c -> p (a c)"), cc_ext[:])
            ss_sb = cpool.tile([P, 2, S], bf16, tag="ss")
            nc.sync.dma_start(ss_sb[:].rearrange("p a c -> p (a c)"), ss_ext[:])
            tri_sb = cpool.tile([P, 2, P], bf16, tag="tri")
            nc.sync.dma_start(tri_sb[:].rearrange("p a c -> p (a c)"), tri_ext[:])
            wo_sb = cpool.tile([CH, D], bf16, tag="wo")
            nc.sync.dma_start(wo_sb[:, 0:512], woT_ext[:, 0:512])
            nc.sync.dma_start(wo_sb[:, 512:1024], woT_ext[:, 512:1024])

            # ---- constants ----
            ident_f = cpool.tile([P, P], f32, tag="identf")
            make_identity(nc, ident_f[:])
            ident = cpool.tile([P, P], bf16, tag="ident")
            nc.vector.tensor_copy(ident[:], ident_f[:])

            ones_f = cpool.tile([P, 64], f32, tag="onesf")
            nc.vector.memset(ones_f[:], 1.0)
            ones_b = cpool.tile([P, 64], bf16, tag="onesb")
            nc.vector.tensor_copy(ones_b[:], ones_f[:])

            # ---- persistent activation tiles ----
            qkT = big.tile([P, 2, NROWS], bf16, tag="qkT")  # [:,0,:]=q [:,1,:]=k
            yT = big.tile([P, NROWS], bf16, tag="yT")
            # per head: [ones | 63 pad | 64 v-dims] = 128 cols, so the
            # softmax denominator lands on psum partition 0 (canonical for
            # gpsimd partition_broadcast) and y-rows sit at partitions
            # 64-127 (DVE ops allow 64-partition APs only at base 0/64)
            v_sb = big.tile([P, KSUB, 256], bf16, tag="v")

            nc.vector.tensor_copy(
                v_sb[:, :, 0:129:128].rearrange("p a b -> p (a b)"),
                ones_b[:, 0:2 * KSUB])
            nc.vector.memset(v_sb[:, :, 1:64], 0.0)
            nc.vector.memset(v_sb[:, :, 129:192], 0.0)

            def phase_a(rt):
                """projections + RoPE for row block rt (512 rows)."""
                sl = slice(rt * RB, (rt + 1) * RB)
                pos = slice((rt % QT_PER_B) * RB, (rt % QT_PER_B + 1) * RB)
                xt = xTb[rt]
                # q|k projection into the two banks of one psum tile
                pqk = psum.tile([P, 1024], f32, tag="big2", name=f"pqk{rt}")
                for d in range(DSUB):
                    nc.tensor.matmul(pqk[:, 0:512], wq_sb[:, d], xt[:, d],
                                     start=(d == 0), stop=(d == DSUB - 1))
                for d in range(DSUB):
                    nc.tensor.matmul(pqk[:, 512:1024], wk_sb[:, d], xt[:, d],
                                     start=(d == 0), stop=(d == DSUB - 1))
                praw = work.tile([P, 2, RB], bf16, tag="praw")
                nc.scalar.activation(praw[:, 0], pqk[:, 0:512],
                                     mybir.ActivationFunctionType.Identity,
                                     bias=bq_sb[:, 0:1], scale=1.0)
                nc.scalar.activation(praw[:, 1], pqk[:, 512:1024],
                                     mybir.ActivationFunctionType.Identity,
                                     bias=bk_sb[:, 0:1], scale=1.0)
                # RoPE: qkT[:, :, sl] = praw*cc + swap32(praw)*ss
                xsw = work.tile([P, 2, RB], bf16, tag="xsw")
                for hh in range(2):
                    b0 = hh * 64
                    nc.sync.dma_start(xsw[b0:b0 + 32], praw[b0 + 32:b0 + 64])
                    nc.sync.dma_start(xsw[b0 + 32:b0 + 64], praw[b0:b0 + 32])
                t1 = work.tile([P, 2, RB], bf16, tag="ropet1")
                nc.vector.tensor_mul(t1[:], praw[:], cc_sb[:, :, pos])
                t2 = work.tile([P, 2, RB], bf16, tag="ropet2")
                nc.vector.tensor_mul(t2[:], xsw[:], ss_sb[:, :, pos])
                nc.vector.tensor_add(qkT[:, :, sl], t1[:], t2[:])

                # V: project, then PE-transpose to natural [k, chan] layout
                pv_ = psum.tile([P, 1024], f32, tag="big2", name=f"pv_{rt}")
                for d in range(DSUB):
                    nc.tensor.matmul(pv_[:, 0:512], wv_sb[:, d], xt[:, d],
                                     start=(d == 0), stop=(d == DSUB - 1))
                vr = work.tile([P, RB], bf16, tag="vraw")
                nc.scalar.activation(vr[:], pv_[:, 0:512],
                                     mybir.ActivationFunctionType.Identity,
                                     bias=bv_sb[:, 0:1], scale=1.0)
                tpv = psumpv.tile([P, 512], bf16, tag="pv", name=f"tpv{rt}")
                for rc in range(4):
                    nc.tensor.transpose(tpv[:, rc * P:(rc + 1) * P],
                                        vr[:, rc * P:(rc + 1) * P], ident[:])
                tpv_v = tpv[:].rearrange("p (k h c) -> p k h c", k=4, h=2)
                vdst = (v_sb[:, rt * 4:(rt + 1) * 4, :]
                        .rearrange("p k (h c) -> p k h c", h=2))
                for hh in range(2):
                    nc.vector.tensor_copy(vdst[:, :, hh, 64:128],
                                          tpv_v[:, :, hh, :])

            def phase_c(b, qt):
                """causal attention for q tile (b, qt) -> yT columns."""
                qcols = slice(b * S + qt * RB, b * S + (qt + 1) * RB)
                nks = qt * 4 + 4
                pv0 = psumpv.tile([P, RB], f32, tag="pv", name=f"pv0_{b}_{qt}")
                pv1 = psumpv.tile([P, RB], f32, tag="pv", name=f"pv1_{b}_{qt}")
                pvs = [pv0, pv1]
                for ks in range(nks):
                    kcols = slice(b * S + ks * P, b * S + (ks + 1) * P)
                    ksg = b * (S // P) + ks
                    m = ks - qt * 4
                    j0 = m * P if m >= 1 else 0
                    qv = slice(b * S + qt * RB + j0, b * S + (qt + 1) * RB)
                    st = psum.tile([P, 1024], f32, tag="big2",
                                   name=f"st{b}_{qt}_{ks}")
                    stv = st[:].rearrange("p (h c) -> p h c", h=2)
                    pt = ptp.tile([P, 2, RB], bf16, tag="pt")
                    for h in range(2):
                        hsl = slice(h * 64, (h + 1) * 64)
                        nc.tensor.matmul(st[:, h * 512 + j0:(h + 1) * 512],
                                         qkT[hsl, 1, kcols], qkT[hsl, 0, qv],
                                         start=True, stop=True)
                    nc.scalar.activation(pt[:, :, j0:], stv[:, :, j0:],
                                         mybir.ActivationFunctionType.Exp)
                    if m >= 0:
                        nc.gpsimd.tensor_mul(pt[:, :, j0:j0 + P],
                                             pt[:, :, j0:j0 + P], tri_sb[:])
                    for h in range(2):
                        nc.tensor.matmul(
                            pvs[h][:, j0:], v_sb[:, ksg, h * P:(h + 1) * P],
                            pt[:, h, j0:],
                            start=(ks == 0), stop=(ks == nks - 1))
                # epilogue: yT[:, qcols] = y_h / denom_h (denom = psum row 0)
                dcp = small.tile([1, 1024], f32, tag="dcp")
                nc.scalar.copy(dcp[0:1, 0:512], pv0[0:1, :])
                nc.vector.tensor_copy(dcp[0:1, 512:1024], pv1[0:1, :])
                dn = small.tile([1, 1024], f32, tag="dn")
                nc.vector.reciprocal_approx_fast(dn[:], dcp[:])
                rep = small.tile([P, 1024], f32, tag="rep")
                nc.gpsimd.partition_broadcast(rep[:], dn[:])
                ynorm = small.tile([P, 1024], bf16, tag="ynorm")
                nc.vector.tensor_mul(ynorm[64:128, 0:512], pv0[64:128, :],
                                     rep[64:128, 0:512])
                nc.vector.tensor_mul(ynorm[64:128, 512:1024], pv1[64:128, :],
                                     rep[64:128, 512:1024])
                nc.sync.dma_start(yT[0:64, qcols], ynorm[64:128, 0:512])
                nc.sync.dma_start(yT[64:128, qcols], ynorm[64:128, 512:1024])
                if DBG and b == 0 and qt == 0:
                    nc.sync.dma_start(dbg_dn[:], dn[:])
                    nc.sync.dma_start(dbg_dcp[:], dcp[:])
                    nc.sync.dma_start(dbg_rep[:], rep[0:64, :])
                    nc.sync.dma_start(
                        dbg_qk[:].rearrange("p (a c) -> p a c", a=2),
                        qkT[:, :, 0:512])
                    nc.sync.dma_start(
                        dbg_v[:].rearrange("p (a c) -> p a c", a=4),
                        v_sb[:, 0:4])
                    nc.sync.dma_start(dbg_yt[:], yT[:, 0:512])

            def phase_d(rt):
                """output projection partial for row chunk rt (128 rows)."""
                ob = obp.tile([P, 1024], bf16, tag="ob")
                for ec in range(2):
                    op = psum.tile([P, 512], f32, tag="big2",
                                   name=f"op{rt}_{ec}")
                    nc.tensor.matmul(op[:], yT[:, rt * P:(rt + 1) * P],
                                     wo_sb[:, ec * 512:(ec + 1) * 512],
                                     start=True, stop=True)
                    eng = nc.vector if ec == 0 else nc.scalar
                    if ec == 0:
                        nc.vector.tensor_copy(ob[:, 0:512], op[:])
                    else:
                        nc.scalar.copy(ob[:, 512:1024], op[:])
                nc.sync.dma_start(
                    out_ext[rt * P:(rt + 1) * P, :], ob[:])

            # ---- interleaved emission: A(rt) then C(b,qt) it unlocks ----
            for rt in range(NRB):
                phase_a(rt)
                phase_c(rt // QT_PER_B, rt % QT_PER_B)
                if rt == 4:
                    for rr in range(0, 16):
                        phase_d(rr)
            for rr in range(16, KSUB):
                phase_d(rr)

    nc.finalize()
    return nc


def _host_inputs():
    t = np.arange(32, dtype=np.float64)
    inv_freq = 1.0 / (ROPE_BASE ** (2.0 * t / DH))
    pos = np.arange(S, dtype=np.float64)
    ang = pos[None, :] * inv_freq[:, None]          # [32, S]
    cos32 = np.cos(ang).astype(np.float32)
    sin32 = np.sin(ang).astype(np.float32)
    cc = np.tile(cos32, (4, 1))                     # [128, S]
    ss = np.concatenate([-sin32, sin32, -sin32, sin32], axis=0)  # [128, S]
    cc2 = np.concatenate([cc, cc], axis=1)          # [128, 2S] (q|k dup)
    ss2 = np.concatenate([ss, ss], axis=1)

    ii = np.arange(P)[:, None]
    uu = np.arange(P)[None, :]
    tri = (uu >= ii).astype(np.float32)             # [128, 128]
    tri2 = np.concatenate([tri, tri], axis=1)       # [128, 256]

    perm64 = np.concatenate([np.arange(0, 64, 2), np.arange(1, 64, 2)])
    return cc2, ss2, tri2, perm64


def _in_maps(x, Wq, bq, Wk, bk, Wv, bv, Wo):
    cc2, ss2, tri2, perm64 = _host_inputs()
    x2 = np.ascontiguousarray(x.reshape(NROWS, D))
    # xT block-major: xT[p, rt, d, c] = x[512*rt + c, 128*d + p]
    xT = np.ascontiguousarray(
        x2.reshape(NRB, RB, DSUB, P).transpose(3, 0, 2, 1)
        .reshape(P, NRB * DSUB * RB)).astype(nbf16)
    perm128 = np.concatenate([perm64, perm64 + 64])
    cc2b = cc2.astype(nbf16)
    ss2b = ss2.astype(nbf16)
    tri2b = tri2.astype(nbf16)
    maps = []
    for c in range(8):
        sl = slice(c * CH, (c + 1) * CH)
        maps.append({
            "xT": xT,
            "wqT": np.ascontiguousarray(
                (Wq[sl][perm128] * 0.125).T).astype(nbf16),
            "wkT": np.ascontiguousarray(Wk[sl][perm128].T).astype(nbf16),
            "wvT": np.ascontiguousarray(Wv[sl].T).astype(nbf16),
            "woT": np.ascontiguousarray(Wo[:, sl].T).astype(nbf16),
            "bq": (bq[sl][perm128] * 0.125).reshape(CH, 1).copy(),
            "bk": bk[sl][perm128].reshape(CH, 1).copy(),
            "bv": bv[sl].reshape(CH, 1).copy(),
            "cc2": cc2b, "ss2": ss2b, "tri": tri2b,
        })
    return maps


def kernel(x, Wq, bq, Wk, bk, Wv, bv, Wo, bo):
    x = np.asarray(x, dtype=np.float32)
    Wq = np.asarray(Wq, dtype=np.float32)
    Wk = np.asarray(Wk, dtype=np.float32)
    Wv = np.asarray(Wv, dtype=np.float32)
    Wo = np.asarray(Wo, dtype=np.float32)
    bq = np.asarray(bq, dtype=np.float32)
    bk = np.asarray(bk, dtype=np.float32)
    bv = np.asarray(bv, dtype=np.float32)
    bo = np.asarray(bo, dtype=np.float32)

    if "nc" not in _CACHE:
        _CACHE["nc"] = _build()
    nc = _CACHE["nc"]

    res = run_bass_kernel_spmd(nc, _in_maps(x, Wq, bq, Wk, bk, Wv, bv, Wo),
                               core_ids=list(range(8)))
    out = np.zeros((NROWS, D), dtype=np.float32)
    for r in res.results:
        out += r["out"].astype(np.float32)
    out += bo[None, :]
    return out.reshape(B, S, D)


# revision 22
# speedup vs baseline: 1.2108x; 1.1346x over previous
"""Multi-head attention (B=2, S=2048, D=1024, H=16, causal, interleaved RoPE)
on 8 Trainium2 NeuronCores.

Sharding: tensor-parallel over heads - 2 heads (128 channels) per core.
Each core computes its Q/K/V projections, RoPE, causal attention, and a
row-parallel partial of the output projection; the host sums the bf16
partials in fp32.

All matmuls in bf16 with fp32 PSUM accumulation. Key layout choices:
  * x^T is pre-transposed and cast to bf16 on the host (block-major layout)
    so no on-device transposes are needed for the projections.
  * Q/K projection weights are fed with output channels permuted so each
    head's dims are [evens(32), odds(32)] -> the RoPE pair-swap becomes a
    32-partition-block swap done with SBUF->SBUF DMAs; RoPE itself is three
    bf16 DVE ops on fused [128,2,512] (q|k) tiles.
  * Attention uses the S^T layout: scores psum [k(128part), q(512)] via
    matmul(lhsT=K^T, rhs=Q^T), the two heads concurrent on disjoint PE row
    groups (partitions 0-63 / 64-127) writing the two banks of one
    [128,1024] psum tile; ONE exp over both heads; causal masking is a
    gpsimd multiply of the [128,128] diagonal strip by a triangular const;
    PV via matmul(lhsT=V_aug, rhs=P^T) where V_aug carries a ones column so
    the denominator drops out as psum row 64; 1/denom via fast-approx
    reciprocal straight off the psum row, broadcast across partitions with
    a gpsimd partition_broadcast (PE/ACT untouched).
  * Emission interleaves projection blocks, attention q-tiles, and output-
    projection chunks so the PE stays dense (HAM-warm) and exp overlaps
    projections.
"""

import os
import numpy as np
import ml_dtypes

import concourse.bacc as bacc
import concourse.mybir as mybir
import concourse.tile as tile
from concourse.bass_utils import run_bass_kernel_spmd
from concourse.masks import make_identity

P = 128
B, S, D = 2, 2048, 1024
H, DH = 16, 64
NROWS = B * S            # 4096 flattened rows
CH = 128                 # channels per core (2 heads)
RB = 512                 # row block for projections / q tiles
NRB = NROWS // RB        # 8
DSUB = D // P            # 8 contraction subtiles
KSUB = NROWS // P        # 32 k subtiles (128 rows each)
QT_PER_B = S // RB       # 4 q tiles per batch
ROPE_BASE = 10000.0

f32 = mybir.dt.float32
bf16 = mybir.dt.bfloat16
nbf16 = ml_dtypes.bfloat16

_CACHE = {}


def _build():
    nc = bacc.Bacc("TRN2", target_bir_lowering=False)

    xT_ext = nc.declare_dram_parameter("xT", [P, NRB * DSUB * RB], bf16,
                                       isOutput=False)
    wqT_ext = nc.declare_dram_parameter("wqT", [D, CH], bf16, isOutput=False)
    wkT_ext = nc.declare_dram_parameter("wkT", [D, CH], bf16, isOutput=False)
    wvT_ext = nc.declare_dram_parameter("wvT", [D, CH], bf16, isOutput=False)
    woT_ext = nc.declare_dram_parameter("woT", [CH, D], bf16, isOutput=False)
    bq_ext = nc.declare_dram_parameter("bq", [CH, 1], f32, isOutput=False)
    bk_ext = nc.declare_dram_parameter("bk", [CH, 1], f32, isOutput=False)
    bv_ext = nc.declare_dram_parameter("bv", [CH, 1], f32, isOutput=False)
    cc_ext = nc.declare_dram_parameter("cc2", [P, 2 * S], bf16, isOutput=False)
    ss_ext = nc.declare_dram_parameter("ss2", [P, 2 * S], bf16, isOutput=False)
    tri_ext = nc.declare_dram_parameter("tri", [P, 2 * P], bf16, isOutput=False)
    out_ext = nc.declare_dram_parameter("out", [NROWS, D], bf16, isOutput=True)
    DBG = bool(os.environ.get("KDBG"))
    if DBG:
        dbg_dn = nc.declare_dram_parameter("dbg_dn", [1, 1024], f32,
                                           isOutput=True)
        dbg_dcp = nc.declare_dram_parameter("dbg_dcp", [1, 1024], f32,
                                            isOutput=True)
        dbg_rep = nc.declare_dram_parameter("dbg_rep", [64, 1024], f32,
                                            isOutput=True)
        dbg_qk = nc.declare_dram_parameter("dbg_qk", [P, 1024], bf16,
                                           isOutput=True)
        dbg_v = nc.declare_dram_parameter("dbg_v", [P, 1024], bf16,
                                          isOutput=True)
        dbg_yt = nc.declare_dram_parameter("dbg_yt", [P, 512], bf16,
                                           isOutput=True)

    with tile.TileContext(nc) as tc:
        with (
            tc.tile_pool(name="const", bufs=1) as cpool,
            tc.tile_pool(name="xpool", bufs=NRB) as xpool,
            tc.tile_pool(name="big", bufs=1) as big,
            tc.tile_pool(name="work", bufs=2) as work,
            tc.tile_pool(name="ptp", bufs=4) as ptp,
            tc.tile_pool(name="small", bufs=2) as small,
            tc.tile_pool(name="obp", bufs=3) as obp,
            tc.tile_pool(name="psum", bufs=2, space="PSUM") as psum,
            tc.tile_pool(name="psumpv", bufs=4, space="PSUM") as psumpv,
        ):
            # ---- input DMAs (weights first: block 0 can start ~5us in) ----
            wq_sb = cpool.tile([P, DSUB, CH], bf16, tag="wq")
            wk_sb = cpool.tile([P, DSUB, CH], bf16, tag="wk")
            wv_sb = cpool.tile([P, DSUB, CH], bf16, tag="wv")
            nc.sync.dma_start(wq_sb[:],
                              wqT_ext[:].rearrange("(d p) c -> p d c", p=P))
            nc.sync.dma_start(wk_sb[:],
                              wkT_ext[:].rearrange("(d p) c -> p d c", p=P))
            nc.sync.dma_start(wv_sb[:],
                              wvT_ext[:].rearrange("(d p) c -> p d c", p=P))
            xTb = []
            for rt in range(NRB):
                xt = xpool.tile([P, DSUB, RB], bf16, tag="xT",
                                name=f"xT{rt}")
                nc.sync.dma_start(
                    xt[:].rearrange("p d c -> p (d c)"),
                    xT_ext[:, rt * DSUB * RB:(rt + 1) * DSUB * RB])
                xTb.append(xt)
            bq_sb = cpool.tile([CH, 1], f32, tag="bq")
            nc.sync.dma_start(bq_sb[:], bq_ext[:])
            bk_sb = cpool.tile([CH, 1], f32, tag="bk")
            nc.sync.dma_start(bk_sb[:], bk_ext[:])
            bv_sb = cpool.tile([CH, 1], f32, tag="bv")
            nc.sync.dma_start(bv_sb[:], bv_ext[:])
            cc_sb = cpool.tile([P, 2, S], bf16, tag="cc")
            nc.sync.dma_start(cc_sb[:].rearrange("p a c -> p (a c)"), cc_ext[:])
            ss_sb = cpool.tile([P, 2, S], bf16, tag="ss")
            nc.sync.dma_start(ss_sb[:].rearrange("p a c -> p (a c)"), ss_ext[:])
            tri_sb = cpool.tile([P, 2, P], bf16, tag="tri")
            nc.sync.dma_start(tri_sb[:].rearrange("p a c -> p (a c)"), tri_ext[:])
            wo_sb = cpool.tile([CH, D], bf16, tag="wo")
            nc.sync.dma_start(wo_sb[:, 0:512], woT_ext[:, 0:512])
            nc.sync.dma_start(wo_sb[:, 512:1024], woT_ext[:, 512:1024])

            # ---- constants ----
            ident_f = cpool.tile([P, P], f32, tag="identf")
            make_identity(nc, ident_f[:])
            ident = cpool.tile([P, P], bf16, tag="ident")
            nc.vector.tensor_copy(ident[:], ident_f[:])

            ones_f = cpool.tile([P, P], f32, tag="onesf")
            nc.vector.memset(ones_f[:], 1.0)
            ones_b = cpool.tile([P, P], bf16, tag="onesb")
            nc.vector.tensor_copy(ones_b[:], ones_f[:])

            # ---- persistent activation tiles ----
            qkT = big.tile([P, 2, NROWS], bf16, tag="qkT")  # [:,0,:]=q [:,1,:]=k
            yT = big.tile([P, NROWS], bf16, tag="yT")
            # per head: [ones | 63 pad | 64 v-dims] = 128 cols, so the
            # softmax denominator lands on psum partition 0 (canonical for
            # gpsimd partition_broadcast) and y-rows sit at partitions
            # 64-127 (DVE ops allow 64-partition APs only at base 0/64)
            v_sb = big.tile([P, KSUB, 256], bf16, tag="v")

            nc.vector.tensor_copy(
                v_sb[:, :, 0:129:128].rearrange("p a b -> p (a b)"),
                ones_b[:, 0:2 * KSUB])
            nc.vector.memset(v_sb[:, :, 1:64], 0.0)
            nc.vector.memset(v_sb[:, :, 129:192], 0.0)

            def phase_a(rt):
                """projections + RoPE for row block rt (512 rows)."""
                sl = slice(rt * RB, (rt + 1) * RB)
                pos = slice((rt % QT_PER_B) * RB, (rt % QT_PER_B + 1) * RB)
                xt = xTb[rt]
                # q|k projection into the two banks of one psum tile
                pqk = psum.tile([P, 1024], f32, tag="big2", name=f"pqk{rt}")
                for d in range(DSUB):
                    nc.tensor.matmul(pqk[:, 0:512], wq_sb[:, d], xt[:, d],
                                     start=(d == 0), stop=(d == DSUB - 1))
                for d in range(DSUB):
                    nc.tensor.matmul(pqk[:, 512:1024], wk_sb[:, d], xt[:, d],
                                     start=(d == 0), stop=(d == DSUB - 1))
                praw = work.tile([P, 2, RB], bf16, tag="praw")
                nc.scalar.activation(praw[:, 0], pqk[:, 0:512],
                                     mybir.ActivationFunctionType.Identity,
                                     bias=bq_sb[:, 0:1], scale=1.0)
                nc.scalar.activation(praw[:, 1], pqk[:, 512:1024],
                                     mybir.ActivationFunctionType.Identity,
                                     bias=bk_sb[:, 0:1], scale=1.0)
                # RoPE: qkT[:, :, sl] = praw*cc + swap32(praw)*ss
                xsw = work.tile([P, 2, RB], bf16, tag="xsw")
                for hh in range(2):
                    b0 = hh * 64
                    nc.sync.dma_start(xsw[b0:b0 + 32], praw[b0 + 32:b0 + 64])
                    nc.sync.dma_start(xsw[b0 + 32:b0 + 64], praw[b0:b0 + 32])
                t1 = work.tile([P, 2, RB], bf16, tag="ropet1")
                nc.vector.tensor_mul(t1[:], praw[:], cc_sb[:, :, pos])
                t2 = work.tile([P, 2, RB], bf16, tag="ropet2")
                nc.vector.tensor_mul(t2[:], xsw[:], ss_sb[:, :, pos])
                nc.vector.tensor_add(qkT[:, :, sl], t1[:], t2[:])

                # V: project, then PE-transpose to natural [k, chan] layout
                pv_ = psum.tile([P, 1024], f32, tag="big2", name=f"pv_{rt}")
                for d in range(DSUB):
                    nc.tensor.matmul(pv_[:, 0:512], wv_sb[:, d], xt[:, d],
                                     start=(d == 0), stop=(d == DSUB - 1))
                vr = work.tile([P, RB], bf16, tag="vraw")
                nc.scalar.activation(vr[:], pv_[:, 0:512],
                                     mybir.ActivationFunctionType.Identity,
                                     bias=bv_sb[:, 0:1], scale=1.0)
                tpv = psumpv.tile([P, 512], bf16, tag="pv", name=f"tpv{rt}")
                for rc in range(4):
                    nc.tensor.transpose(tpv[:, rc * P:(rc + 1) * P],
                                        vr[:, rc * P:(rc + 1) * P], ident[:])
                tpv_v = tpv[:].rearrange("p (k h c) -> p k h c", k=4, h=2)
                vdst = (v_sb[:, rt * 4:(rt + 1) * 4, :]
                        .rearrange("p k (h c) -> p k h c", h=2))
                for hh in range(2):
                    nc.vector.tensor_copy(vdst[:, :, hh, 64:128],
                                          tpv_v[:, :, hh, :])

            def phase_c(b, qt):
                """causal attention for q tile (b, qt) -> yT columns."""
                qcols = slice(b * S + qt * RB, b * S + (qt + 1) * RB)
                nks = qt * 4 + 4
                pv0 = psumpv.tile([P, RB], f32, tag="pv", name=f"pv0_{b}_{qt}")
                pv1 = psumpv.tile([P, RB], f32, tag="pv", name=f"pv1_{b}_{qt}")
                pvs = [pv0, pv1]
                for ks in range(nks):
                    kcols = slice(b * S + ks * P, b * S + (ks + 1) * P)
                    ksg = b * (S // P) + ks
                    m = ks - qt * 4
                    j0 = m * P if m >= 1 else 0
                    qv = slice(b * S + qt * RB + j0, b * S + (qt + 1) * RB)
                    st = psum.tile([P, 1024], f32, tag="big2",
                                   name=f"st{b}_{qt}_{ks}")
                    stv = st[:].rearrange("p (h c) -> p h c", h=2)
                    pt = ptp.tile([P, 2, RB], bf16, tag="pt")
                    for h in range(2):
                        hsl = slice(h * 64, (h + 1) * 64)
                        nc.tensor.matmul(st[:, h * 512 + j0:(h + 1) * 512],
                                         qkT[hsl, 1, kcols], qkT[hsl, 0, qv],
                                         start=True, stop=True)
                    nc.scalar.activation(pt[:, :, j0:], stv[:, :, j0:],
                                         mybir.ActivationFunctionType.Exp)
                    if m >= 0:
                        nc.vector.tensor_mul(pt[:, :, j0:j0 + P],
                                             pt[:, :, j0:j0 + P], tri_sb[:])
                    for h in range(2):
                        nc.tensor.matmul(
                            pvs[h][:, j0:], v_sb[:, ksg, h * P:(h + 1) * P],
                            pt[:, h, j0:],
                            start=(ks == 0), stop=(ks == nks - 1))
                # epilogue: yT[:, qcols] = y_h / denom_h (denom = psum row 0)
                dcp = small.tile([1, 1024], f32, tag="dcp")
                nc.scalar.copy(dcp[0:1, 0:512], pv0[0:1, :])
                nc.vector.tensor_copy(dcp[0:1, 512:1024], pv1[0:1, :])
                dn = small.tile([1, 1024], f32, tag="dn")
                nc.vector.reciprocal_approx_fast(dn[:], dcp[:])
                dnb = small.tile([1, 1024], bf16, tag="dnb")
                nc.scalar.copy(dnb[:], dn[:])
                rp = psum.tile([P, 1024], f32, tag="big2", name=f"rp{b}_{qt}")
                nc.tensor.matmul(rp[:, 0:512], ones_b[0:1, :],
                                 dnb[0:1, 0:512], start=True, stop=True)
                nc.tensor.matmul(rp[:, 512:1024], ones_b[0:1, :],
                                 dnb[0:1, 512:1024], start=True, stop=True)
                rep = small.tile([P, 1024], f32, tag="rep")
                nc.vector.tensor_copy(rep[:], rp[:])
                ynorm = small.tile([P, 1024], bf16, tag="ynorm")
                nc.vector.tensor_mul(ynorm[64:128, 0:512], pv0[64:128, :],
                                     rep[64:128, 0:512])
                nc.vector.tensor_mul(ynorm[64:128, 512:1024], pv1[64:128, :],
                                     rep[64:128, 512:1024])
                nc.sync.dma_start(yT[0:64, qcols], ynorm[64:128, 0:512])
                nc.sync.dma_start(yT[64:128, qcols], ynorm[64:128, 512:1024])
                if DBG and b == 0 and qt == 0:
                    nc.sync.dma_start(dbg_dn[:], dn[:])
                    nc.sync.dma_start(dbg_dcp[:], dcp[:])
                    nc.sync.dma_start(dbg_rep[:], rep[0:64, :])
                    nc.sync.dma_start(
                        dbg_qk[:].rearrange("p (a c) -> p a c", a=2),
                        qkT[:, :, 0:512])
                    nc.sync.dma_start(
                        dbg_v[:].rearrange("p (a c) -> p a c", a=4),
                        v_sb[:, 0:4])
                    nc.sync.dma_start(dbg_yt[:], yT[:, 0:512])

            def phase_d(rt):
                """output projection partial for row chunk rt (128 rows)."""
                ob = obp.tile([P, 1024], bf16, tag="ob")
                for ec in range(2):
                    op = psum.tile([P, 512], f32, tag="big2",
                                   name=f"op{rt}_{ec}")
                    nc.tensor.matmul(op[:], yT[:, rt * P:(rt + 1) * P],
                                     wo_sb[:, ec * 512:(ec + 1) * 512],
                                     start=True, stop=True)
                    eng = nc.vector if ec == 0 else nc.scalar
                    if ec == 0:
                        nc.vector.tensor_copy(ob[:, 0:512], op[:])
                    else:
                        nc.scalar.copy(ob[:, 512:1024], op[:])
                nc.sync.dma_start(
                    out_ext[rt * P:(rt + 1) * P, :], ob[:])

            # ---- interleaved emission: A(rt) then C(b,qt) it unlocks ----
            for rt in range(NRB):
                phase_a(rt)
                phase_c(rt // QT_PER_B, rt % QT_PER_B)
                if rt == 4:
                    for rr in range(0, 16):
                        phase_d(rr)
            for rr in range(16, KSUB):
                phase_d(rr)

    nc.finalize()
    return nc


def _host_inputs():
    t = np.arange(32, dtype=np.float64)
    inv_freq = 1.0 / (ROPE_BASE ** (2.0 * t / DH))
    pos = np.arange(S, dtype=np.float64)
    ang = pos[None, :] * inv_freq[:, None]          # [32, S]
    cos32 = np.cos(ang).astype(np.float32)
    sin32 = np.sin(ang).astype(np.float32)
    cc = np.tile(cos32, (4, 1))                     # [128, S]
    ss = np.concatenate([-sin32, sin32, -sin32, sin32], axis=0)  # [128, S]
    cc2 = np.concatenate([cc, cc], axis=1)          # [128, 2S] (q|k dup)
    ss2 = np.concatenate([ss, ss], axis=1)

    ii = np.arange(P)[:, None]
    uu = np.arange(P)[None, :]
    tri = (uu >= ii).astype(np.float32)             # [128, 128]
    tri2 = np.concatenate([tri, tri], axis=1)       # [128, 256]

    perm64 = np.concatenate([np.arange(0, 64, 2), np.arange(1, 64, 2)])
    return cc2, ss2, tri2, perm64


def _in_maps(x, Wq, bq, Wk, bk, Wv, bv, Wo):
    cc2, ss2, tri2, perm64 = _host_inputs()
    x2 = np.ascontiguousarray(x.reshape(NROWS, D))
    # xT block-major: xT[p, rt, d, c] = x[512*rt + c, 128*d + p]
    xT = np.ascontiguousarray(
        x2.reshape(NRB, RB, DSUB, P).transpose(3, 0, 2, 1)
        .reshape(P, NRB * DSUB * RB)).astype(nbf16)
    perm128 = np.concatenate([perm64, perm64 + 64])
    cc2b = cc2.astype(nbf16)
    ss2b = ss2.astype(nbf16)
    tri2b = tri2.astype(nbf16)
    maps = []
    for c in range(8):
        sl = slice(c * CH, (c + 1) * CH)
        maps.append({
            "xT": xT,
            "wqT": np.ascontiguousarray(
                (Wq[sl][perm128] * 0.125).T).astype(nbf16),
            "wkT": np.ascontiguousarray(Wk[sl][perm128].T).astype(nbf16),
            "wvT": np.ascontiguousarray(Wv[sl].T).astype(nbf16),
            "woT": np.ascontiguousarray(Wo[:, sl].T).astype(nbf16),
            "bq": (bq[sl][perm128] * 0.125).reshape(CH, 1).copy(),
            "bk": bk[sl][perm128].reshape(CH, 1).copy(),
            "bv": bv[sl].reshape(CH, 1).copy(),
            "cc2": cc2b, "ss2": ss2b, "tri": tri2b,
        })
    return maps


def kernel(x, Wq, bq, Wk, bk, Wv, bv, Wo, bo):
    x = np.asarray(x, dtype=np.float32)
    Wq = np.asarray(Wq, dtype=np.float32)
    Wk = np.asarray(Wk, dtype=np.float32)
    Wv = np.asarray(Wv, dtype=np.float32)
    Wo = np.asarray(Wo, dtype=np.float32)
    bq = np.asarray(bq, dtype=np.float32)
    bk = np.asarray(bk, dtype=np.float32)
    bv = np.asarray(bv, dtype=np.float32)
    bo = np.asarray(bo, dtype=np.float32)

    if "nc" not in _CACHE:
        _CACHE["nc"] = _build()
    nc = _CACHE["nc"]

    res = run_bass_kernel_spmd(nc, _in_maps(x, Wq, bq, Wk, bk, Wv, bv, Wo),
                               core_ids=list(range(8)))
    out = np.zeros((NROWS, D), dtype=np.float32)
    for r in res.results:
        out += r["out"].astype(np.float32)
    out += bo[None, :]
    return out.reshape(B, S, D)


# revision 24
# speedup vs baseline: 1.4855x; 1.2268x over previous
"""Multi-head attention (B=2, S=2048, D=1024, H=16, causal, interleaved RoPE)
on 8 Trainium2 NeuronCores.

Sharding: tensor-parallel over heads - 2 heads (128 channels) per core.
Each core computes its Q/K/V projections, RoPE, causal attention, and a
row-parallel partial of the output projection; the host sums the bf16
partials in fp32.

All matmuls in bf16 with fp32 PSUM accumulation. Key layout choices:
  * x^T is pre-transposed and cast to bf16 on the host (block-major layout)
    so no on-device transposes are needed for the projections.
  * Q/K projection weights are fed with output channels permuted so each
    head's dims are [evens(32), odds(32)] -> the RoPE pair-swap becomes a
    32-partition-block swap done with SBUF->SBUF DMAs; RoPE itself is three
    bf16 DVE ops on fused [128,2,512] (q|k) tiles.
  * Attention uses the S^T layout: scores psum [k(128part), q(512)] via
    matmul(lhsT=K^T, rhs=Q^T), the two heads concurrent on disjoint PE row
    groups (partitions 0-63 / 64-127) writing the two banks of one
    [128,1024] psum tile; ONE exp over both heads; causal masking is a
    gpsimd multiply of the [128,128] diagonal strip by a triangular const;
    PV via matmul(lhsT=V_aug, rhs=P^T) where V_aug carries a ones column so
    the denominator drops out as psum row 64; 1/denom via fast-approx
    reciprocal straight off the psum row, broadcast across partitions with
    a gpsimd partition_broadcast (PE/ACT untouched).
  * Emission interleaves projection blocks, attention q-tiles, and output-
    projection chunks so the PE stays dense (HAM-warm) and exp overlaps
    projections.
"""

import os
import numpy as np
import ml_dtypes

import concourse.bacc as bacc
import concourse.mybir as mybir
import concourse.tile as tile
from concourse.bass_utils import run_bass_kernel_spmd
from concourse.masks import make_identity

P = 128
B, S, D = 2, 2048, 1024
H, DH = 16, 64
NROWS = B * S            # 4096 flattened rows
CH = 128                 # channels per core (2 heads)
RB = 512                 # row block for projections / q tiles
NRB = NROWS // RB        # 8
DSUB = D // P            # 8 contraction subtiles
KSUB = NROWS // P        # 32 k subtiles (128 rows each)
QT_PER_B = S // RB       # 4 q tiles per batch
ROPE_BASE = 10000.0

f32 = mybir.dt.float32
bf16 = mybir.dt.bfloat16
nbf16 = ml_dtypes.bfloat16

_CACHE = {}


def _build():
    nc = bacc.Bacc("TRN2", target_bir_lowering=False)

    xT_ext = nc.declare_dram_parameter("xT", [P, NRB * DSUB * RB], bf16,
                                       isOutput=False)
    wqT_ext = nc.declare_dram_parameter("wqT", [D, CH], bf16, isOutput=False)
    wkT_ext = nc.declare_dram_parameter("wkT", [D, CH], bf16, isOutput=False)
    wvT_ext = nc.declare_dram_parameter("wvT", [D, CH], bf16, isOutput=False)
    woT_ext = nc.declare_dram_parameter("woT", [CH, D], bf16, isOutput=False)
    bq_ext = nc.declare_dram_parameter("bq", [CH, 1], f32, isOutput=False)
    bk_ext = nc.declare_dram_parameter("bk", [CH, 1], f32, isOutput=False)
    bv_ext = nc.declare_dram_parameter("bv", [CH, 1], f32, isOutput=False)
    cc_ext = nc.declare_dram_parameter("cc2", [P, 2 * S], bf16, isOutput=False)
    ss_ext = nc.declare_dram_parameter("ss2", [P, 2 * S], bf16, isOutput=False)
    tri_ext = nc.declare_dram_parameter("tri", [P, 2 * P], bf16, isOutput=False)
    out_ext = nc.declare_dram_parameter("out", [NROWS, D], bf16, isOutput=True)
    DBG = bool(os.environ.get("KDBG"))
    if DBG:
        dbg_dn = nc.declare_dram_parameter("dbg_dn", [1, 1024], f32,
                                           isOutput=True)
        dbg_dcp = nc.declare_dram_parameter("dbg_dcp", [1, 1024], f32,
                                            isOutput=True)
        dbg_rep = nc.declare_dram_parameter("dbg_rep", [64, 1024], f32,
                                            isOutput=True)
        dbg_qk = nc.declare_dram_parameter("dbg_qk", [P, 1024], bf16,
                                           isOutput=True)
        dbg_v = nc.declare_dram_parameter("dbg_v", [P, 1024], bf16,
                                          isOutput=True)
        dbg_yt = nc.declare_dram_parameter("dbg_yt", [P, 512], bf16,
                                           isOutput=True)

    with tile.TileContext(nc) as tc:
        with (
            tc.tile_pool(name="const", bufs=1) as cpool,
            tc.tile_pool(name="xpool", bufs=NRB) as xpool,
            tc.tile_pool(name="big", bufs=1) as big,
            tc.tile_pool(name="work", bufs=2) as work,
            tc.tile_pool(name="ptp", bufs=4) as ptp,
            tc.tile_pool(name="small", bufs=2) as small,
            tc.tile_pool(name="obp", bufs=3) as obp,
            tc.tile_pool(name="psum", bufs=2, space="PSUM") as psum,
            tc.tile_pool(name="psumpv", bufs=4, space="PSUM") as psumpv,
        ):
            # ---- input DMAs (weights first: block 0 can start ~5us in) ----
            wq_sb = cpool.tile([P, DSUB, CH], bf16, tag="wq")
            wk_sb = cpool.tile([P, DSUB, CH], bf16, tag="wk")
            wv_sb = cpool.tile([P, DSUB, CH], bf16, tag="wv")
            nc.sync.dma_start(wq_sb[:],
                              wqT_ext[:].rearrange("(d p) c -> p d c", p=P))
            nc.sync.dma_start(wk_sb[:],
                              wkT_ext[:].rearrange("(d p) c -> p d c", p=P))
            nc.sync.dma_start(wv_sb[:],
                              wvT_ext[:].rearrange("(d p) c -> p d c", p=P))
            bq_sb = cpool.tile([CH, 1], f32, tag="bq")
            nc.sync.dma_start(bq_sb[:], bq_ext[:])
            bk_sb = cpool.tile([CH, 1], f32, tag="bk")
            nc.sync.dma_start(bk_sb[:], bk_ext[:])
            bv_sb = cpool.tile([CH, 1], f32, tag="bv")
            nc.sync.dma_start(bv_sb[:], bv_ext[:])
            cc_sb = cpool.tile([P, 2, S], bf16, tag="cc")
            nc.sync.dma_start(cc_sb[:].rearrange("p a c -> p (a c)"), cc_ext[:])
            ss_sb = cpool.tile([P, 2, S], bf16, tag="ss")
            nc.sync.dma_start(ss_sb[:].rearrange("p a c -> p (a c)"), ss_ext[:])
            tri_sb = cpool.tile([P, 2, P], bf16, tag="tri")
            nc.sync.dma_start(tri_sb[:].rearrange("p a c -> p (a c)"), tri_ext[:])
            wo_sb = cpool.tile([CH, D], bf16, tag="wo")
            nc.sync.dma_start(wo_sb[:, 0:512], woT_ext[:, 0:512])
            nc.sync.dma_start(wo_sb[:, 512:1024], woT_ext[:, 512:1024])
            xTb = []
            for rt in range(NRB):
                xt = xpool.tile([P, DSUB, RB], bf16, tag="xT",
                                name=f"xT{rt}")
                nc.sync.dma_start(
                    xt[:].rearrange("p d c -> p (d c)"),
                    xT_ext[:, rt * DSUB * RB:(rt + 1) * DSUB * RB])
                xTb.append(xt)

            # ---- constants ----
            ident_f = cpool.tile([P, P], f32, tag="identf")
            make_identity(nc, ident_f[:])
            ident = cpool.tile([P, P], bf16, tag="ident")
            nc.vector.tensor_copy(ident[:], ident_f[:])

            ones_f = cpool.tile([P, P], f32, tag="onesf")
            nc.vector.memset(ones_f[:], 1.0)
            ones_b = cpool.tile([P, P], bf16, tag="onesb")
            nc.vector.tensor_copy(ones_b[:], ones_f[:])

            # ---- persistent activation tiles ----
            qkT = big.tile([P, 2, NROWS], bf16, tag="qkT")  # [:,0,:]=q [:,1,:]=k
            yT = big.tile([P, NROWS], bf16, tag="yT")
            # per head: [ones | 63 pad | 64 v-dims] = 128 cols, so the
            # softmax denominator lands on psum partition 0 (canonical for
            # gpsimd partition_broadcast) and y-rows sit at partitions
            # 64-127 (DVE ops allow 64-partition APs only at base 0/64)
            v_sb = big.tile([P, KSUB, 256], bf16, tag="v")

            nc.vector.tensor_copy(
                v_sb[:, :, 0:129:128].rearrange("p a b -> p (a b)"),
                ones_b[:, 0:2 * KSUB])
            nc.vector.memset(v_sb[:, :, 1:64], 0.0)
            nc.vector.memset(v_sb[:, :, 129:192], 0.0)

            def phase_a(rt):
                """projections + RoPE for row block rt (512 rows)."""
                sl = slice(rt * RB, (rt + 1) * RB)
                pos = slice((rt % QT_PER_B) * RB, (rt % QT_PER_B + 1) * RB)
                xt = xTb[rt]
                # q|k projection into the two banks of one psum tile
                pqk = psum.tile([P, 1024], f32, tag="big2", name=f"pqk{rt}")
                for d in range(DSUB):
                    nc.tensor.matmul(pqk[:, 0:512], wq_sb[:, d], xt[:, d],
                                     start=(d == 0), stop=(d == DSUB - 1))
                for d in range(DSUB):
                    nc.tensor.matmul(pqk[:, 512:1024], wk_sb[:, d], xt[:, d],
                                     start=(d == 0), stop=(d == DSUB - 1))
                praw = work.tile([P, 2, RB], bf16, tag="praw")
                nc.vector.tensor_scalar_add(praw[:, 0], pqk[:, 0:512],
                                            bq_sb[:, 0:1])
                nc.vector.tensor_scalar_add(praw[:, 1], pqk[:, 512:1024],
                                            bk_sb[:, 0:1])
                # RoPE: qkT[:, :, sl] = praw*cc + swap32(praw)*ss
                xsw = work.tile([P, 2, RB], bf16, tag="xsw")
                for hh in range(2):
                    b0 = hh * 64
                    nc.sync.dma_start(xsw[b0:b0 + 32], praw[b0 + 32:b0 + 64])
                    nc.sync.dma_start(xsw[b0 + 32:b0 + 64], praw[b0:b0 + 32])
                t1 = work.tile([P, 2, RB], bf16, tag="ropet1")
                nc.vector.tensor_mul(t1[:], praw[:], cc_sb[:, :, pos])
                t2 = work.tile([P, 2, RB], bf16, tag="ropet2")
                nc.vector.tensor_mul(t2[:], xsw[:], ss_sb[:, :, pos])
                nc.vector.tensor_add(qkT[:, :, sl], t1[:], t2[:])

                # V: project, then PE-transpose to natural [k, chan] layout
                pv_ = psum.tile([P, 1024], f32, tag="big2", name=f"pv_{rt}")
                for d in range(DSUB):
                    nc.tensor.matmul(pv_[:, 0:512], wv_sb[:, d], xt[:, d],
                                     start=(d == 0), stop=(d == DSUB - 1))
                vr = work.tile([P, RB], bf16, tag="vraw")
                nc.vector.tensor_scalar_add(vr[:], pv_[:, 0:512],
                                            bv_sb[:, 0:1])
                tpv = psumpv.tile([P, 512], bf16, tag="pv", name=f"tpv{rt}")
                for rc in range(4):
                    nc.tensor.transpose(tpv[:, rc * P:(rc + 1) * P],
                                        vr[:, rc * P:(rc + 1) * P], ident[:])
                tpv_v = tpv[:].rearrange("p (k h c) -> p k h c", k=4, h=2)
                vdst = (v_sb[:, rt * 4:(rt + 1) * 4, :]
                        .rearrange("p k (h c) -> p k h c", h=2))
                for hh in range(2):
                    nc.vector.tensor_copy(vdst[:, :, hh, 64:128],
                                          tpv_v[:, :, hh, :])

            def phase_c(b, qt):
                """causal attention q-tile (b, qt): scores run 2 ks-steps
                ahead of PV so the PE never waits on exp (ACT) in-queue."""
                qcols = slice(b * S + qt * RB, b * S + (qt + 1) * RB)
                nks = qt * 4 + 4
                pv0 = psumpv.tile([P, RB], f32, tag="pv", name=f"pv0_{b}_{qt}")
                pv1 = psumpv.tile([P, RB], f32, tag="pv", name=f"pv1_{b}_{qt}")
                pvs = [pv0, pv1]
                pts = {}

                def j0_of(ks):
                    m = ks - qt * 4
                    return m * P if m >= 1 else 0

                for ks in range(nks):
                    kcols = slice(b * S + ks * P, b * S + (ks + 1) * P)
                    m = ks - qt * 4
                    j0 = j0_of(ks)
                    qv = slice(b * S + qt * RB + j0, b * S + (qt + 1) * RB)
                    st = psum.tile([P, 1024], f32, tag="big2",
                                   name=f"st{b}_{qt}_{ks}")
                    stv = st[:].rearrange("p (h c) -> p h c", h=2)
                    pt = ptp.tile([P, 2, RB], bf16, tag="pt")
                    pts[ks] = pt
                    for h in range(2):
                        hsl = slice(h * 64, (h + 1) * 64)
                        nc.tensor.matmul(st[:, h * 512 + j0:(h + 1) * 512],
                                         qkT[hsl, 1, kcols], qkT[hsl, 0, qv],
                                         start=True, stop=True)
                    nc.scalar.activation(pt[:, :, j0:], stv[:, :, j0:],
                                         mybir.ActivationFunctionType.Exp)
                    if m >= 0:
                        nc.vector.tensor_mul(pt[:, :, j0:j0 + P],
                                             pt[:, :, j0:j0 + P], tri_sb[:])
                    if ks >= 2:
                        kk = ks - 2
                        jj = j0_of(kk)
                        ptk = pts.pop(kk)
                        for h in range(2):
                            nc.tensor.matmul(
                                pvs[h][:, jj:],
                                v_sb[:, b * (S // P) + kk, h * P:(h + 1) * P],
                                ptk[:, h, jj:],
                                start=(kk == 0), stop=(kk == nks - 1))
                for kk in (nks - 2, nks - 1):
                    jj = j0_of(kk)
                    ptk = pts.pop(kk)
                    for h in range(2):
                        nc.tensor.matmul(
                            pvs[h][:, jj:],
                            v_sb[:, b * (S // P) + kk, h * P:(h + 1) * P],
                            ptk[:, h, jj:],
                            start=(kk == 0), stop=(kk == nks - 1))
                return (b, qt, qcols, pv0, pv1)

            def phase_c_epi1(state):
                """denominator reciprocal chain (ACT/DVE only, no PE)."""
                b, qt, qcols, pv0, pv1 = state
                dcp = small.tile([1, 1024], f32, tag="dcp")
                nc.scalar.copy(dcp[0:1, 0:512], pv0[0:1, :])
                nc.vector.tensor_copy(dcp[0:1, 512:1024], pv1[0:1, :])
                dn = small.tile([1, 1024], f32, tag="dn")
                nc.vector.reciprocal_approx_fast(dn[:], dcp[:])
                dnb = small.tile([1, 1024], bf16, tag="dnb")
                nc.scalar.copy(dnb[:], dn[:])
                return state + (dnb,)

            def phase_c_epi2(state):
                """broadcast 1/denom + normalize into yT."""
                b, qt, qcols, pv0, pv1, dnb = state
                rp = psum.tile([P, 1024], f32, tag="big2", name=f"rp{b}_{qt}")
                nc.tensor.matmul(rp[:, 0:512], ones_b[0:1, :],
                                 dnb[0:1, 0:512], start=True, stop=True)
                nc.tensor.matmul(rp[:, 512:1024], ones_b[0:1, :],
                                 dnb[0:1, 512:1024], start=True, stop=True)
                rep = small.tile([P, 1024], f32, tag="rep")
                nc.vector.tensor_copy(rep[:], rp[:])
                ynorm = small.tile([P, 1024], bf16, tag="ynorm")
                nc.vector.tensor_mul(ynorm[64:128, 0:512], pv0[64:128, :],
                                     rep[64:128, 0:512])
                nc.vector.tensor_mul(ynorm[64:128, 512:1024], pv1[64:128, :],
                                     rep[64:128, 512:1024])
                nc.sync.dma_start(yT[0:64, qcols], ynorm[64:128, 0:512])
                nc.sync.dma_start(yT[64:128, qcols], ynorm[64:128, 512:1024])

            def phase_d(rt):
                """output projection partial for row chunk rt (128 rows)."""
                ob = obp.tile([P, 1024], bf16, tag="ob")
                for ec in range(2):
                    op = psum.tile([P, 512], f32, tag="big2",
                                   name=f"op{rt}_{ec}")
                    nc.tensor.matmul(op[:], yT[:, rt * P:(rt + 1) * P],
                                     wo_sb[:, ec * 512:(ec + 1) * 512],
                                     start=True, stop=True)
                    eng = nc.vector if ec == 0 else nc.scalar
                    if ec == 0:
                        nc.vector.tensor_copy(ob[:, 0:512], op[:])
                    else:
                        nc.scalar.copy(ob[:, 512:1024], op[:])
                nc.gpsimd.dma_start(
                    out_ext[rt * P:(rt + 1) * P, :], ob[:])

            # ---- interleaved emission: A(rt) then C(b,qt) it unlocks;
            # the previous C's epilogue straddles A so its ACT/DVE chain
            # resolves while the PE chews A's projection matmuls ----
            prev = None
            for rt in range(NRB):
                if prev is not None:
                    prev = phase_c_epi1(prev)
                phase_a(rt)
                if prev is not None:
                    phase_c_epi2(prev)
                prev = phase_c(rt // QT_PER_B, rt % QT_PER_B)
                if rt == 5:
                    for rr in range(0, 16):
                        phase_d(rr)
            prev = phase_c_epi1(prev)
            phase_c_epi2(prev)
            for rr in range(16, KSUB):
                phase_d(rr)

    nc.finalize()
    return nc


def _host_inputs():
    t = np.arange(32, dtype=np.float64)
    inv_freq = 1.0 / (ROPE_BASE ** (2.0 * t / DH))
    pos = np.arange(S, dtype=np.float64)
    ang = pos[None, :] * inv_freq[:, None]          # [32, S]
    cos32 = np.cos(ang).astype(np.float32)
    sin32 = np.sin(ang).astype(np.float32)
    cc = np.tile(cos32, (4, 1))                     # [128, S]
    ss = np.concatenate([-sin32, sin32, -sin32, sin32], axis=0)  # [128, S]
    cc2 = np.concatenate([cc, cc], axis=1)          # [128, 2S] (q|k dup)
    ss2 = np.concatenate([ss, ss], axis=1)

    ii = np.arange(P)[:, None]
    uu = np.arange(P)[None, :]
    tri = (uu >= ii).astype(np.float32)             # [128, 128]
    tri2 = np.concatenate([tri, tri], axis=1)       # [128, 256]

    perm64 = np.concatenate([np.arange(0, 64, 2), np.arange(1, 64, 2)])
    return cc2, ss2, tri2, perm64


def _in_maps(x, Wq, bq, Wk, bk, Wv, bv, Wo):
    cc2, ss2, tri2, perm64 = _host_inputs()
    x2 = np.ascontiguousarray(x.reshape(NROWS, D))
    # xT block-major: xT[p, rt, d, c] = x[512*rt + c, 128*d + p]
    xT = np.ascontiguousarray(
        x2.reshape(NRB, RB, DSUB, P).transpose(3, 0, 2, 1)
        .reshape(P, NRB * DSUB * RB)).astype(nbf16)
    perm128 = np.concatenate([perm64, perm64 + 64])
    cc2b = cc2.astype(nbf16)
    ss2b = ss2.astype(nbf16)
    tri2b = tri2.astype(nbf16)
    maps = []
    for c in range(8):
        sl = slice(c * CH, (c + 1) * CH)
        maps.append({
            "xT": xT,
            "wqT": np.ascontiguousarray(
                (Wq[sl][perm128] * 0.125).T).astype(nbf16),
            "wkT": np.ascontiguousarray(Wk[sl][perm128].T).astype(nbf16),
            "wvT": np.ascontiguousarray(Wv[sl].T).astype(nbf16),
            "woT": np.ascontiguousarray(Wo[:, sl].T).astype(nbf16),
            "bq": (bq[sl][perm128] * 0.125).reshape(CH, 1).copy(),
            "bk": bk[sl][perm128].reshape(CH, 1).copy(),
            "bv": bv[sl].reshape(CH, 1).copy(),
            "cc2": cc2b, "ss2": ss2b, "tri": tri2b,
        })
    return maps


def kernel(x, Wq, bq, Wk, bk, Wv, bv, Wo, bo):
    x = np.asarray(x, dtype=np.float32)
    Wq = np.asarray(Wq, dtype=np.float32)
    Wk = np.asarray(Wk, dtype=np.float32)
    Wv = np.asarray(Wv, dtype=np.float32)
    Wo = np.asarray(Wo, dtype=np.float32)
    bq = np.asarray(bq, dtype=np.float32)
    bk = np.asarray(bk, dtype=np.float32)
    bv = np.asarray(bv, dtype=np.float32)
    bo = np.asarray(bo, dtype=np.float32)

    if "nc" not in _CACHE:
        _CACHE["nc"] = _build()
    nc = _CACHE["nc"]

    res = run_bass_kernel_spmd(nc, _in_maps(x, Wq, bq, Wk, bk, Wv, bv, Wo),
                               core_ids=list(range(8)))
    out = np.zeros((NROWS, D), dtype=np.float32)
    for r in res.results:
        out += r["out"].astype(np.float32)
    out += bo[None, :]
    return out.reshape(B, S, D)


# revision 26
# speedup vs baseline: 1.7485x; 1.1771x over previous
"""Multi-head attention (B=2, S=2048, D=1024, H=16, causal, interleaved RoPE)
on 8 Trainium2 NeuronCores.

Sharding: tensor-parallel over heads - 2 heads (128 channels) per core.
Each core computes its Q/K/V projections, RoPE, causal attention, and a
row-parallel partial of the output projection; the host sums the bf16
partials in fp32.

All matmuls in bf16 with fp32 PSUM accumulation. Key layout choices:
  * x^T is pre-transposed and cast to bf16 on the host (block-major layout)
    so no on-device transposes are needed for the projections.
  * Q/K projection weights are fed with output channels permuted so each
    head's dims are [evens(32), odds(32)] -> the RoPE pair-swap becomes a
    32-partition-block swap done with SBUF->SBUF DMAs; RoPE itself is three
    bf16 DVE ops on fused [128,2,512] (q|k) tiles.
  * Attention uses the S^T layout: scores psum [k(128part), q(512)] via
    matmul(lhsT=K^T, rhs=Q^T), the two heads concurrent on disjoint PE row
    groups (partitions 0-63 / 64-127) writing the two banks of one
    [128,1024] psum tile; ONE exp over both heads; causal masking is a
    gpsimd multiply of the [128,128] diagonal strip by a triangular const;
    PV via matmul(lhsT=V_aug, rhs=P^T) where V_aug carries a ones column so
    the denominator drops out as psum row 64; 1/denom via fast-approx
    reciprocal straight off the psum row, broadcast across partitions with
    a gpsimd partition_broadcast (PE/ACT untouched).
  * Emission interleaves projection blocks, attention q-tiles, and output-
    projection chunks so the PE stays dense (HAM-warm) and exp overlaps
    projections.
"""

import os
import numpy as np
import ml_dtypes

import concourse.bacc as bacc
import concourse.mybir as mybir
import concourse.tile as tile
from concourse.bass_utils import run_bass_kernel_spmd
from concourse.masks import make_identity

P = 128
B, S, D = 2, 2048, 1024
H, DH = 16, 64
NROWS = B * S            # 4096 flattened rows
CH = 128                 # channels per core (2 heads)
RB = 512                 # row block for projections / q tiles
NRB = NROWS // RB        # 8
DSUB = D // P            # 8 contraction subtiles
KSUB = NROWS // P        # 32 k subtiles (128 rows each)
QT_PER_B = S // RB       # 4 q tiles per batch
ROPE_BASE = 10000.0

f32 = mybir.dt.float32
bf16 = mybir.dt.bfloat16
nbf16 = ml_dtypes.bfloat16

_CACHE = {}


def _build():
    nc = bacc.Bacc("TRN2", target_bir_lowering=False)

    xT_ext = nc.declare_dram_parameter("xT", [P, NRB * DSUB * RB], bf16,
                                       isOutput=False)
    wqT_ext = nc.declare_dram_parameter("wqT", [D, CH], bf16, isOutput=False)
    wkT_ext = nc.declare_dram_parameter("wkT", [D, CH], bf16, isOutput=False)
    wvT_ext = nc.declare_dram_parameter("wvT", [D, CH], bf16, isOutput=False)
    woT_ext = nc.declare_dram_parameter("woT", [CH, D], bf16, isOutput=False)
    bq_ext = nc.declare_dram_parameter("bq", [CH, 1], f32, isOutput=False)
    bk_ext = nc.declare_dram_parameter("bk", [CH, 1], f32, isOutput=False)
    bv_ext = nc.declare_dram_parameter("bv", [CH, 1], f32, isOutput=False)
    cc_ext = nc.declare_dram_parameter("cc2", [P, 2 * S], bf16, isOutput=False)
    ss_ext = nc.declare_dram_parameter("ss2", [P, 2 * S], bf16, isOutput=False)
    tri_ext = nc.declare_dram_parameter("tri", [P, 2 * P], bf16, isOutput=False)
    psw_ext = nc.declare_dram_parameter("pswm", [P, P], bf16, isOutput=False)
    out_ext = nc.declare_dram_parameter("out", [NROWS, D], bf16, isOutput=True)
    DBG = bool(os.environ.get("KDBG"))
    if DBG:
        dbg_dn = nc.declare_dram_parameter("dbg_dn", [1, 1024], f32,
                                           isOutput=True)
        dbg_dcp = nc.declare_dram_parameter("dbg_dcp", [1, 1024], f32,
                                            isOutput=True)
        dbg_rep = nc.declare_dram_parameter("dbg_rep", [64, 1024], f32,
                                            isOutput=True)
        dbg_qk = nc.declare_dram_parameter("dbg_qk", [P, 1024], bf16,
                                           isOutput=True)
        dbg_v = nc.declare_dram_parameter("dbg_v", [P, 1024], bf16,
                                          isOutput=True)
        dbg_yt = nc.declare_dram_parameter("dbg_yt", [P, 512], bf16,
                                           isOutput=True)

    with tile.TileContext(nc) as tc:
        with (
            tc.tile_pool(name="const", bufs=1) as cpool,
            tc.tile_pool(name="xpool", bufs=NRB) as xpool,
            tc.tile_pool(name="big", bufs=1) as big,
            tc.tile_pool(name="work", bufs=2) as work,
            tc.tile_pool(name="ptp", bufs=4) as ptp,
            tc.tile_pool(name="small", bufs=2) as small,
            tc.tile_pool(name="obp", bufs=3) as obp,
            tc.tile_pool(name="psum", bufs=2, space="PSUM") as psum,
            tc.tile_pool(name="psumpv", bufs=4, space="PSUM") as psumpv,
        ):
            # ---- input DMAs, ordered so block 0's deps land first ----
            wq_sb = cpool.tile([P, DSUB, CH], bf16, tag="wq")
            wk_sb = cpool.tile([P, DSUB, CH], bf16, tag="wk")
            wv_sb = cpool.tile([P, DSUB, CH], bf16, tag="wv")
            nc.sync.dma_start(wq_sb[:],
                              wqT_ext[:].rearrange("(d p) c -> p d c", p=P))
            nc.sync.dma_start(wk_sb[:],
                              wkT_ext[:].rearrange("(d p) c -> p d c", p=P))
            xTb = []

            def load_xt(rt):
                xt = xpool.tile([P, DSUB, RB], bf16, tag="xT", name=f"xT{rt}")
                nc.sync.dma_start(
                    xt[:].rearrange("p d c -> p (d c)"),
                    xT_ext[:, rt * DSUB * RB:(rt + 1) * DSUB * RB])
                xTb.append(xt)

            load_xt(0)
            nc.sync.dma_start(wv_sb[:],
                              wvT_ext[:].rearrange("(d p) c -> p d c", p=P))
            psw_sb = cpool.tile([P, P], bf16, tag="pswm")
            nc.sync.dma_start(psw_sb[:], psw_ext[:])
            bq_sb = cpool.tile([CH, 1], f32, tag="bq")
            nc.sync.dma_start(bq_sb[:], bq_ext[:])
            bk_sb = cpool.tile([CH, 1], f32, tag="bk")
            nc.sync.dma_start(bk_sb[:], bk_ext[:])
            bv_sb = cpool.tile([CH, 1], f32, tag="bv")
            nc.sync.dma_start(bv_sb[:], bv_ext[:])
            cc_sb = cpool.tile([P, 2, S], bf16, tag="cc")
            nc.sync.dma_start(cc_sb[:].rearrange("p a c -> p (a c)"), cc_ext[:])
            ss_sb = cpool.tile([P, 2, S], bf16, tag="ss")
            nc.sync.dma_start(ss_sb[:].rearrange("p a c -> p (a c)"), ss_ext[:])
            load_xt(1)
            tri_sb = cpool.tile([P, 2, P], bf16, tag="tri")
            nc.sync.dma_start(tri_sb[:].rearrange("p a c -> p (a c)"), tri_ext[:])
            for rt in range(2, NRB):
                load_xt(rt)
            wo_sb = cpool.tile([CH, D], bf16, tag="wo")
            nc.sync.dma_start(wo_sb[:, 0:512], woT_ext[:, 0:512])
            nc.sync.dma_start(wo_sb[:, 512:1024], woT_ext[:, 512:1024])

            # ---- constants ----
            ident_f = cpool.tile([P, P], f32, tag="identf")
            make_identity(nc, ident_f[:])
            ident = cpool.tile([P, P], bf16, tag="ident")
            nc.vector.tensor_copy(ident[:], ident_f[:])

            ones_f = cpool.tile([P, P], f32, tag="onesf")
            nc.vector.memset(ones_f[:], 1.0)
            ones_b = cpool.tile([P, P], bf16, tag="onesb")
            nc.vector.tensor_copy(ones_b[:], ones_f[:])

            # ---- persistent activation tiles ----
            qkT = big.tile([P, 2, NROWS], bf16, tag="qkT")  # [:,0,:]=q [:,1,:]=k
            yT = big.tile([P, NROWS], bf16, tag="yT")
            # per head: [ones | 63 pad | 64 v-dims] = 128 cols, so the
            # softmax denominator lands on psum partition 0 (canonical for
            # gpsimd partition_broadcast) and y-rows sit at partitions
            # 64-127 (DVE ops allow 64-partition APs only at base 0/64)
            v_sb = big.tile([P, KSUB, 256], bf16, tag="v")

            nc.vector.tensor_copy(
                v_sb[:, :, 0:129:128].rearrange("p a b -> p (a b)"),
                ones_b[:, 0:2 * KSUB])
            nc.vector.memset(v_sb[:, :, 1:64], 0.0)
            nc.vector.memset(v_sb[:, :, 129:192], 0.0)

            def phase_a(rt):
                """projections + RoPE for row block rt (512 rows)."""
                sl = slice(rt * RB, (rt + 1) * RB)
                pos = slice((rt % QT_PER_B) * RB, (rt % QT_PER_B + 1) * RB)
                xt = xTb[rt]
                # q|k projection into the two banks of one psum tile
                pqk = psum.tile([P, 1024], f32, tag="big2", name=f"pqk{rt}")
                for d in range(DSUB):
                    nc.tensor.matmul(pqk[:, 0:512], wq_sb[:, d], xt[:, d],
                                     start=(d == 0), stop=(d == DSUB - 1))
                for d in range(DSUB):
                    nc.tensor.matmul(pqk[:, 512:1024], wk_sb[:, d], xt[:, d],
                                     start=(d == 0), stop=(d == DSUB - 1))
                praw = work.tile([P, 2, RB], bf16, tag="praw")
                nc.vector.tensor_scalar_add(praw[:, 0], pqk[:, 0:512],
                                            bq_sb[:, 0:1])
                nc.vector.tensor_scalar_add(praw[:, 1], pqk[:, 512:1024],
                                            bk_sb[:, 0:1])
                # RoPE: qkT[:, :, sl] = praw*cc + swap32(praw)*ss.
                # swap32 is a partition permutation -> one PE matmul with a
                # 0/1 permutation matrix (keeps the PE queue dense; no DMAs)
                psw = psum.tile([P, 1024], f32, tag="big2", name=f"psw{rt}")
                prflat = praw[:].rearrange("p a c -> p (a c)")
                nc.tensor.matmul(psw[:, 0:512], psw_sb[:], prflat[:, 0:512],
                                 start=True, stop=True)
                nc.tensor.matmul(psw[:, 512:1024], psw_sb[:],
                                 prflat[:, 512:1024], start=True, stop=True)
                t1 = work.tile([P, 2, RB], bf16, tag="ropet1")
                nc.vector.tensor_mul(t1[:], praw[:], cc_sb[:, :, pos])
                t2 = work.tile([P, 2, RB], bf16, tag="ropet2")
                nc.vector.tensor_mul(
                    t2[:], psw[:].rearrange("p (a c) -> p a c", a=2),
                    ss_sb[:, :, pos])
                nc.vector.tensor_add(qkT[:, :, sl], t1[:], t2[:])

                # V: project, then PE-transpose to natural [k, chan] layout
                pv_ = psum.tile([P, 1024], f32, tag="big2", name=f"pv_{rt}")
                for d in range(DSUB):
                    nc.tensor.matmul(pv_[:, 0:512], wv_sb[:, d], xt[:, d],
                                     start=(d == 0), stop=(d == DSUB - 1))
                vr = work.tile([P, RB], bf16, tag="vraw")
                nc.vector.tensor_scalar_add(vr[:], pv_[:, 0:512],
                                            bv_sb[:, 0:1])
                tpv = psumpv.tile([P, 512], bf16, tag="pv", name=f"tpv{rt}")
                for rc in range(4):
                    nc.tensor.transpose(tpv[:, rc * P:(rc + 1) * P],
                                        vr[:, rc * P:(rc + 1) * P], ident[:])
                tpv_v = tpv[:].rearrange("p (k h c) -> p k h c", k=4, h=2)
                vdst = (v_sb[:, rt * 4:(rt + 1) * 4, :]
                        .rearrange("p k (h c) -> p k h c", h=2))
                for hh in range(2):
                    nc.vector.tensor_copy(vdst[:, :, hh, 64:128],
                                          tpv_v[:, :, hh, :])

            def phase_c(b, qt):
                """causal attention q-tile (b, qt): scores run 2 ks-steps
                ahead of PV so the PE never waits on exp (ACT) in-queue."""
                qcols = slice(b * S + qt * RB, b * S + (qt + 1) * RB)
                nks = qt * 4 + 4
                pv0 = psumpv.tile([P, RB], f32, tag="pv", name=f"pv0_{b}_{qt}")
                pv1 = psumpv.tile([P, RB], f32, tag="pv", name=f"pv1_{b}_{qt}")
                pvs = [pv0, pv1]
                pts = {}

                def j0_of(ks):
                    m = ks - qt * 4
                    return m * P if m >= 1 else 0

                for ks in range(nks):
                    kcols = slice(b * S + ks * P, b * S + (ks + 1) * P)
                    m = ks - qt * 4
                    j0 = j0_of(ks)
                    qv = slice(b * S + qt * RB + j0, b * S + (qt + 1) * RB)
                    st = psum.tile([P, 1024], f32, tag="big2",
                                   name=f"st{b}_{qt}_{ks}")
                    stv = st[:].rearrange("p (h c) -> p h c", h=2)
                    pt = ptp.tile([P, 2, RB], bf16, tag="pt")
                    pts[ks] = pt
                    for h in range(2):
                        hsl = slice(h * 64, (h + 1) * 64)
                        nc.tensor.matmul(st[:, h * 512 + j0:(h + 1) * 512],
                                         qkT[hsl, 1, kcols], qkT[hsl, 0, qv],
                                         start=True, stop=True)
                    nc.scalar.activation(pt[:, :, j0:], stv[:, :, j0:],
                                         mybir.ActivationFunctionType.Exp)
                    if m >= 0:
                        nc.vector.tensor_mul(pt[:, :, j0:j0 + P],
                                             pt[:, :, j0:j0 + P], tri_sb[:])
                    if ks >= 2:
                        kk = ks - 2
                        jj = j0_of(kk)
                        ptk = pts.pop(kk)
                        for h in range(2):
                            nc.tensor.matmul(
                                pvs[h][:, jj:],
                                v_sb[:, b * (S // P) + kk, h * P:(h + 1) * P],
                                ptk[:, h, jj:],
                                start=(kk == 0), stop=(kk == nks - 1))
                for kk in (nks - 2, nks - 1):
                    jj = j0_of(kk)
                    ptk = pts.pop(kk)
                    for h in range(2):
                        nc.tensor.matmul(
                            pvs[h][:, jj:],
                            v_sb[:, b * (S // P) + kk, h * P:(h + 1) * P],
                            ptk[:, h, jj:],
                            start=(kk == 0), stop=(kk == nks - 1))
                return (b, qt, qcols, pv0, pv1)

            def phase_c_epi1(state):
                """denominator reciprocal chain (ACT/DVE only, no PE)."""
                b, qt, qcols, pv0, pv1 = state
                dcp = small.tile([1, 1024], f32, tag="dcp")
                nc.scalar.copy(dcp[0:1, 0:512], pv0[0:1, :])
                nc.vector.tensor_copy(dcp[0:1, 512:1024], pv1[0:1, :])
                dn = small.tile([1, 1024], f32, tag="dn")
                nc.vector.reciprocal_approx_fast(dn[:], dcp[:])
                dnb = small.tile([1, 1024], bf16, tag="dnb")
                nc.scalar.copy(dnb[:], dn[:])
                return state + (dnb,)

            def phase_c_epi2(state):
                """broadcast 1/denom + normalize into yT."""
                b, qt, qcols, pv0, pv1, dnb = state
                rp = psum.tile([P, 1024], f32, tag="big2", name=f"rp{b}_{qt}")
                nc.tensor.matmul(rp[:, 0:512], ones_b[0:1, :],
                                 dnb[0:1, 0:512], start=True, stop=True)
                nc.tensor.matmul(rp[:, 512:1024], ones_b[0:1, :],
                                 dnb[0:1, 512:1024], start=True, stop=True)
                rep = small.tile([P, 1024], f32, tag="rep")
                nc.vector.tensor_copy(rep[:], rp[:])
                ynorm = small.tile([P, 1024], bf16, tag="ynorm")
                nc.vector.tensor_mul(ynorm[64:128, 0:512], pv0[64:128, :],
                                     rep[64:128, 0:512])
                nc.vector.tensor_mul(ynorm[64:128, 512:1024], pv1[64:128, :],
                                     rep[64:128, 512:1024])
                nc.sync.dma_start(yT[0:64, qcols], ynorm[64:128, 0:512])
                nc.sync.dma_start(yT[64:128, qcols], ynorm[64:128, 512:1024])

            def phase_d(rt):
                """output projection partial for row chunk rt (128 rows)."""
                ob = obp.tile([P, 1024], bf16, tag="ob")
                for ec in range(2):
                    op = psum.tile([P, 512], f32, tag="big2",
                                   name=f"op{rt}_{ec}")
                    nc.tensor.matmul(op[:], yT[:, rt * P:(rt + 1) * P],
                                     wo_sb[:, ec * 512:(ec + 1) * 512],
                                     start=True, stop=True)
                    eng = nc.vector if ec == 0 else nc.scalar
                    if ec == 0:
                        nc.vector.tensor_copy(ob[:, 0:512], op[:])
                    else:
                        nc.scalar.copy(ob[:, 512:1024], op[:])
                nc.sync.dma_start(
                    out_ext[rt * P:(rt + 1) * P, :], ob[:])

            # ---- interleaved emission: A(rt) then C(b,qt) it unlocks;
            # the previous C's epilogue straddles A so its ACT/DVE chain
            # resolves while the PE chews A's projection matmuls ----
            prev = None
            for rt in range(NRB):
                if prev is not None:
                    prev = phase_c_epi1(prev)
                phase_a(rt)
                if prev is not None:
                    phase_c_epi2(prev)
                prev = phase_c(rt // QT_PER_B, rt % QT_PER_B)
                if rt == 5:
                    for rr in range(0, 16):
                        phase_d(rr)
            prev = phase_c_epi1(prev)
            phase_c_epi2(prev)
            for rr in range(16, KSUB):
                phase_d(rr)

    nc.finalize()
    return nc


def _host_inputs():
    t = np.arange(32, dtype=np.float64)
    inv_freq = 1.0 / (ROPE_BASE ** (2.0 * t / DH))
    pos = np.arange(S, dtype=np.float64)
    ang = pos[None, :] * inv_freq[:, None]          # [32, S]
    cos32 = np.cos(ang).astype(np.float32)
    sin32 = np.sin(ang).astype(np.float32)
    cc = np.tile(cos32, (4, 1))                     # [128, S]
    ss = np.concatenate([-sin32, sin32, -sin32, sin32], axis=0)  # [128, S]
    cc2 = np.concatenate([cc, cc], axis=1)          # [128, 2S] (q|k dup)
    ss2 = np.concatenate([ss, ss], axis=1)

    ii = np.arange(P)[:, None]
    uu = np.arange(P)[None, :]
    tri = (uu >= ii).astype(np.float32)             # [128, 128]
    tri2 = np.concatenate([tri, tri], axis=1)       # [128, 256]

    perm64 = np.concatenate([np.arange(0, 64, 2), np.arange(1, 64, 2)])
    return cc2, ss2, tri2, perm64


def _in_maps(x, Wq, bq, Wk, bk, Wv, bv, Wo):
    cc2, ss2, tri2, perm64 = _host_inputs()
    # swap32 permutation matrix: psw[i,:] = praw[src(i),:], src(i) = i^32
    # within each 64-block -> pswm[k, m] = 1 iff k == src(m)
    pswm = np.zeros((P, P), dtype=np.float32)
    for m_ in range(P):
        k_ = (m_ & ~63) | ((m_ + 32) & 63)
        pswm[k_, m_] = 1.0
    pswm = pswm.astype(nbf16)
    x2 = np.ascontiguousarray(x.reshape(NROWS, D))
    # xT block-major: xT[p, rt, d, c] = x[512*rt + c, 128*d + p]
    xT = np.ascontiguousarray(
        x2.reshape(NRB, RB, DSUB, P).transpose(3, 0, 2, 1)
        .reshape(P, NRB * DSUB * RB)).astype(nbf16)
    perm128 = np.concatenate([perm64, perm64 + 64])
    cc2b = cc2.astype(nbf16)
    ss2b = ss2.astype(nbf16)
    tri2b = tri2.astype(nbf16)
    maps = []
    for c in range(8):
        sl = slice(c * CH, (c + 1) * CH)
        maps.append({
            "xT": xT,
            "wqT": np.ascontiguousarray(
                (Wq[sl][perm128] * 0.125).T).astype(nbf16),
            "wkT": np.ascontiguousarray(Wk[sl][perm128].T).astype(nbf16),
            "wvT": np.ascontiguousarray(Wv[sl].T).astype(nbf16),
            "woT": np.ascontiguousarray(Wo[:, sl].T).astype(nbf16),
            "bq": (bq[sl][perm128] * 0.125).reshape(CH, 1).copy(),
            "bk": bk[sl][perm128].reshape(CH, 1).copy(),
            "bv": bv[sl].reshape(CH, 1).copy(),
            "cc2": cc2b, "ss2": ss2b, "tri": tri2b, "pswm": pswm,
        })
    return maps


def kernel(x, Wq, bq, Wk, bk, Wv, bv, Wo, bo):
    x = np.asarray(x, dtype=np.float32)
    Wq = np.asarray(Wq, dtype=np.float32)
    Wk = np.asarray(Wk, dtype=np.float32)
    Wv = np.asarray(Wv, dtype=np.float32)
    Wo = np.asarray(Wo, dtype=np.float32)
    bq = np.asarray(bq, dtype=np.float32)
    bk = np.asarray(bk, dtype=np.float32)
    bv = np.asarray(bv, dtype=np.float32)
    bo = np.asarray(bo, dtype=np.float32)

    if "nc" not in _CACHE:
        _CACHE["nc"] = _build()
    nc = _CACHE["nc"]

    res = run_bass_kernel_spmd(nc, _in_maps(x, Wq, bq, Wk, bk, Wv, bv, Wo),
                               core_ids=list(range(8)))
    out = np.zeros((NROWS, D), dtype=np.float32)
    for r in res.results:
        out += r["out"].astype(np.float32)
    out += bo[None, :]
    return out.reshape(B, S, D)


# revision 27
# speedup vs baseline: 1.7815x; 1.0188x over previous
"""Multi-head attention (B=2, S=2048, D=1024, H=16, causal, interleaved RoPE)
on 8 Trainium2 NeuronCores.

Sharding: tensor-parallel over heads - 2 heads (128 channels) per core.
Each core computes its Q/K/V projections, RoPE, causal attention, and a
row-parallel partial of the output projection; the host sums the bf16
partials in fp32.

All matmuls in bf16 with fp32 PSUM accumulation. Key structure:
  * x^T is pre-transposed and cast to bf16 on the host (block-major layout)
    so no on-device transposes are needed for the projections.
  * Q/K projection weights are host-permuted so each head's dims are
    [evens(32), odds(32)]; the RoPE pair-swap is then a 32-partition-block
    permutation done with ONE PE matmul against a 0/1 permutation matrix.
  * Attention uses the S^T layout: scores psum [k(128part), q(512)] via
    matmul(lhsT=K^T, rhs=Q^T), the two heads concurrent on disjoint PE row
    groups writing the two banks of one [128,1024] psum tile; ONE exp over
    both heads; causal masking multiplies only the [128,2x128] diagonal
    strip by a triangular constant (DVE). PV via matmul(lhsT=V_aug, rhs=P^T)
    with V_aug = [ones | 63 zero-pad | v dims] per head: the softmax
    denominator lands on psum partition 0 and y-rows on partitions 64-127
    (legal DVE base). 1/denom via fast-approx reciprocal, broadcast over
    partitions with a K=1 PE matmul.
  * Fine-grained software pipelining: projection blocks, softmax epilogues
    and output-projection chunks are emitted as small filler bundles between
    the ks-steps of the attention loop, so the PE queue always holds
    independent work while ACT chews exp (keeps HAM at 2.4 GHz).
"""

from collections import deque

import numpy as np
import ml_dtypes

import concourse.bacc as bacc
import concourse.mybir as mybir
import concourse.tile as tile
from concourse.bass_utils import run_bass_kernel_spmd
from concourse.masks import make_identity

P = 128
B, S, D = 2, 2048, 1024
H, DH = 16, 64
NROWS = B * S            # 4096 flattened rows
CH = 128                 # channels per core (2 heads)
RB = 512                 # row block for projections / q tiles
NRB = NROWS // RB        # 8
DSUB = D // P            # 8 contraction subtiles
KSUB = NROWS // P        # 32 k subtiles (128 rows each)
QT_PER_B = S // RB       # 4 q tiles per batch
ROPE_BASE = 10000.0

f32 = mybir.dt.float32
bf16 = mybir.dt.bfloat16
nbf16 = ml_dtypes.bfloat16

_CACHE = {}


def _build():
    nc = bacc.Bacc("TRN2", target_bir_lowering=False)

    xT_ext = nc.declare_dram_parameter("xT", [P, NRB * DSUB * RB], bf16,
                                       isOutput=False)
    wqT_ext = nc.declare_dram_parameter("wqT", [D, CH], bf16, isOutput=False)
    wkT_ext = nc.declare_dram_parameter("wkT", [D, CH], bf16, isOutput=False)
    wvT_ext = nc.declare_dram_parameter("wvT", [D, CH], bf16, isOutput=False)
    woT_ext = nc.declare_dram_parameter("woT", [CH, D], bf16, isOutput=False)
    bq_ext = nc.declare_dram_parameter("bq", [CH, 1], f32, isOutput=False)
    bk_ext = nc.declare_dram_parameter("bk", [CH, 1], f32, isOutput=False)
    bv_ext = nc.declare_dram_parameter("bv", [CH, 1], f32, isOutput=False)
    cc_ext = nc.declare_dram_parameter("cc2", [P, 2 * S], bf16, isOutput=False)
    ss_ext = nc.declare_dram_parameter("ss2", [P, 2 * S], bf16, isOutput=False)
    tri_ext = nc.declare_dram_parameter("tri", [P, 2 * P], bf16, isOutput=False)
    psw_ext = nc.declare_dram_parameter("pswm", [P, P], bf16, isOutput=False)
    out_ext = nc.declare_dram_parameter("out", [NROWS, D], bf16, isOutput=True)

    with tile.TileContext(nc) as tc:
        with (
            tc.tile_pool(name="const", bufs=1) as cpool,
            tc.tile_pool(name="xpool", bufs=NRB) as xpool,
            tc.tile_pool(name="big", bufs=1) as big,
            tc.tile_pool(name="work", bufs=2) as work,
            tc.tile_pool(name="ptp", bufs=4) as ptp,
            tc.tile_pool(name="small", bufs=2) as small,
            tc.tile_pool(name="obp", bufs=3) as obp,
            tc.tile_pool(name="psum", bufs=2, space="PSUM") as psum,
            tc.tile_pool(name="psacc", bufs=2, space="PSUM") as psacc,
        ):
            # ---- input DMAs, ordered so block 0's deps land first ----
            wq_sb = cpool.tile([P, DSUB, CH], bf16, tag="wq")
            wk_sb = cpool.tile([P, DSUB, CH], bf16, tag="wk")
            wv_sb = cpool.tile([P, DSUB, CH], bf16, tag="wv")
            nc.sync.dma_start(wq_sb[:],
                              wqT_ext[:].rearrange("(d p) c -> p d c", p=P))
            nc.sync.dma_start(wk_sb[:],
                              wkT_ext[:].rearrange("(d p) c -> p d c", p=P))
            xTb = []

            def load_xt(rt):
                xt = xpool.tile([P, DSUB, RB], bf16, tag="xT", name=f"xT{rt}")
                nc.sync.dma_start(
                    xt[:].rearrange("p d c -> p (d c)"),
                    xT_ext[:, rt * DSUB * RB:(rt + 1) * DSUB * RB])
                xTb.append(xt)

            load_xt(0)
            nc.sync.dma_start(wv_sb[:],
                              wvT_ext[:].rearrange("(d p) c -> p d c", p=P))
            psw_sb = cpool.tile([P, P], bf16, tag="pswm")
            nc.sync.dma_start(psw_sb[:], psw_ext[:])
            bq_sb = cpool.tile([CH, 1], f32, tag="bq")
            nc.sync.dma_start(bq_sb[:], bq_ext[:])
            bk_sb = cpool.tile([CH, 1], f32, tag="bk")
            nc.sync.dma_start(bk_sb[:], bk_ext[:])
            bv_sb = cpool.tile([CH, 1], f32, tag="bv")
            nc.sync.dma_start(bv_sb[:], bv_ext[:])
            cc_sb = cpool.tile([P, 2, S], bf16, tag="cc")
            nc.sync.dma_start(cc_sb[:].rearrange("p a c -> p (a c)"), cc_ext[:])
            ss_sb = cpool.tile([P, 2, S], bf16, tag="ss")
            nc.sync.dma_start(ss_sb[:].rearrange("p a c -> p (a c)"), ss_ext[:])
            load_xt(1)
            tri_sb = cpool.tile([P, 2, P], bf16, tag="tri")
            nc.sync.dma_start(tri_sb[:].rearrange("p a c -> p (a c)"),
                              tri_ext[:])
            for rt in range(2, NRB):
                load_xt(rt)
            wo_sb = cpool.tile([CH, D], bf16, tag="wo")
            nc.sync.dma_start(wo_sb[:, 0:512], woT_ext[:, 0:512])
            nc.sync.dma_start(wo_sb[:, 512:1024], woT_ext[:, 512:1024])

            # ---- constants ----
            ones_f = cpool.tile([P, P], f32, tag="onesf")
            nc.vector.memset(ones_f[:], 1.0)
            ones_b = cpool.tile([P, P], bf16, tag="onesb")
            nc.vector.tensor_copy(ones_b[:], ones_f[:])
            ident_f = cpool.tile([P, P], f32, tag="identf")
            make_identity(nc, ident_f[:])
            ident = cpool.tile([P, P], bf16, tag="ident")
            nc.vector.tensor_copy(ident[:], ident_f[:])

            # ---- persistent activation tiles ----
            qkT = big.tile([P, 2, NROWS], bf16, tag="qkT")  # [:,0]=q [:,1]=k
            yT = big.tile([P, NROWS], bf16, tag="yT")
            # per head: [ones | 63 pad | 64 v-dims] = 128 cols
            v_sb = big.tile([P, KSUB, 256], bf16, tag="v")

            nc.vector.tensor_copy(
                v_sb[:, :, 0:129:128].rearrange("p a b -> p (a b)"),
                ones_b[:, 0:2 * KSUB])
            nc.vector.memset(v_sb[:, :, 1:64], 0.0)
            nc.vector.memset(v_sb[:, :, 129:192], 0.0)

            # ---------- phase A (projections + RoPE) as filler chunks ------
            def a_chunks(rt):
                sl = slice(rt * RB, (rt + 1) * RB)
                pos = slice((rt % QT_PER_B) * RB, (rt % QT_PER_B + 1) * RB)
                xt = xTb[rt]
                st_ = {}

                def a1():
                    pqk = psacc.tile([P, 1024], f32, tag="acc",
                                     name=f"pqk{rt}")
                    st_["pqk"] = pqk
                    for d in range(4):
                        nc.tensor.matmul(pqk[:, 0:512], wq_sb[:, d], xt[:, d],
                                         start=(d == 0), stop=False)

                def a2():
                    pqk = st_["pqk"]
                    for d in range(4, 8):
                        nc.tensor.matmul(pqk[:, 0:512], wq_sb[:, d], xt[:, d],
                                         start=False, stop=(d == 7))
                    praw = work.tile([P, 2, RB], bf16, tag="praw")
                    st_["praw"] = praw
                    nc.vector.tensor_scalar_add(praw[:, 0], pqk[:, 0:512],
                                                bq_sb[:, 0:1])

                def a3():
                    pqk = st_["pqk"]
                    for d in range(4):
                        nc.tensor.matmul(pqk[:, 512:1024], wk_sb[:, d],
                                         xt[:, d], start=(d == 0), stop=False)

                def a4():
                    pqk = st_["pqk"]
                    for d in range(4, 8):
                        nc.tensor.matmul(pqk[:, 512:1024], wk_sb[:, d],
                                         xt[:, d], start=False, stop=(d == 7))
                    nc.vector.tensor_scalar_add(st_["praw"][:, 1],
                                                pqk[:, 512:1024],
                                                bk_sb[:, 0:1])

                def a5():
                    # swap32 via PE permutation, overwriting the pqk banks
                    pqk, praw = st_["pqk"], st_["praw"]
                    prflat = praw[:].rearrange("p a c -> p (a c)")
                    nc.tensor.matmul(pqk[:, 0:512], psw_sb[:],
                                     prflat[:, 0:512], start=True, stop=True)
                    nc.tensor.matmul(pqk[:, 512:1024], psw_sb[:],
                                     prflat[:, 512:1024], start=True,
                                     stop=True)
                    t1 = work.tile([P, 2, RB], bf16, tag="ropet1")
                    st_["t1"] = t1
                    nc.vector.tensor_mul(t1[:], praw[:], cc_sb[:, :, pos])

                def a6():
                    pqk = st_["pqk"]
                    t2 = work.tile([P, 2, RB], bf16, tag="ropet2")
                    nc.vector.tensor_mul(
                        t2[:], pqk[:].rearrange("p (a c) -> p a c", a=2),
                        ss_sb[:, :, pos])
                    nc.vector.tensor_add(qkT[:, :, sl], st_["t1"][:], t2[:])

                def a7():
                    pqk = st_["pqk"]
                    for d in range(4):
                        nc.tensor.matmul(pqk[:, 0:512], wv_sb[:, d], xt[:, d],
                                         start=(d == 0), stop=False)

                def a8():
                    pqk = st_["pqk"]
                    for d in range(4, 8):
                        nc.tensor.matmul(pqk[:, 0:512], wv_sb[:, d], xt[:, d],
                                         start=False, stop=(d == 7))
                    vr = work.tile([P, RB], bf16, tag="vraw")
                    st_["vr"] = vr
                    nc.vector.tensor_scalar_add(vr[:], pqk[:, 0:512],
                                                bv_sb[:, 0:1])

                def a9():
                    vr = st_["vr"]
                    tpv = psum.tile([P, 512], bf16, tag="st", name=f"tpv{rt}")
                    for rc in range(4):
                        nc.tensor.transpose(tpv[:, rc * P:(rc + 1) * P],
                                            vr[:, rc * P:(rc + 1) * P],
                                            ident[:])
                    tpv_v = tpv[:].rearrange("p (k h c) -> p k h c", k=4, h=2)
                    vdst = (v_sb[:, rt * 4:(rt + 1) * 4, :]
                            .rearrange("p k (h c) -> p k h c", h=2))
                    for hh in range(2):
                        nc.vector.tensor_copy(vdst[:, :, hh, 64:128],
                                              tpv_v[:, :, hh, :])

                return [a1, a2, a3, a4, a5, a6, a7, a8, a9]

            # ---------- softmax epilogue as filler chunks ----------
            def epi_chunks(state):
                b, qt, qcols, pvm = state
                st_ = {}

                def e1():
                    dcp = small.tile([1, 1024], f32, tag="dcp")
                    nc.scalar.copy(dcp[:], pvm[0:1, :])
                    dn = small.tile([1, 1024], f32, tag="dn")
                    nc.vector.reciprocal_approx_fast(dn[:], dcp[:])
                    dnb = small.tile([1, 1024], bf16, tag="dnb")
                    nc.scalar.copy(dnb[:], dn[:])
                    st_["dnb"] = dnb

                def e2():
                    dnb = st_["dnb"]
                    rp = psum.tile([P, 1024], f32, tag="st",
                                   name=f"rp{b}_{qt}")
                    nc.tensor.matmul(rp[:, 0:512], ones_b[0:1, :],
                                     dnb[0:1, 0:512], start=True, stop=True)
                    nc.tensor.matmul(rp[:, 512:1024], ones_b[0:1, :],
                                     dnb[0:1, 512:1024], start=True,
                                     stop=True)
                    rep = small.tile([P, 1024], f32, tag="rep")
                    nc.vector.tensor_copy(rep[:], rp[:])
                    st_["rep"] = rep

                def e3():
                    rep = st_["rep"]
                    ynorm = small.tile([P, 1024], bf16, tag="ynorm")
                    nc.vector.tensor_mul(ynorm[64:128, 0:512],
                                         pvm[64:128, 0:512],
                                         rep[64:128, 0:512])
                    nc.vector.tensor_mul(ynorm[64:128, 512:1024],
                                         pvm[64:128, 512:1024],
                                         rep[64:128, 512:1024])
                    nc.sync.dma_start(yT[0:64, qcols], ynorm[64:128, 0:512])
                    nc.sync.dma_start(yT[64:128, qcols],
                                      ynorm[64:128, 512:1024])

                return [e1, e2, e3]

            # ---------- phase D (output projection) as filler chunks ------
            def d_chunk(rt):
                def d1():
                    op = psum.tile([P, 1024], f32, tag="st", name=f"op{rt}")
                    for ec in range(2):
                        nc.tensor.matmul(op[:, ec * 512:(ec + 1) * 512],
                                         yT[:, rt * P:(rt + 1) * P],
                                         wo_sb[:, ec * 512:(ec + 1) * 512],
                                         start=True, stop=True)
                    ob = obp.tile([P, 1024], bf16, tag="ob")
                    if rt % 2 == 0:
                        nc.vector.tensor_copy(ob[:], op[:])
                    else:
                        nc.scalar.copy(ob[:], op[:])
                    nc.sync.dma_start(out_ext[rt * P:(rt + 1) * P, :], ob[:])
                return d1

            # ---------- attention q-tile with fillers ----------
            def phase_c(b, qt, fillers):
                qcols = slice(b * S + qt * RB, b * S + (qt + 1) * RB)
                nks = qt * 4 + 4
                pvm = psacc.tile([P, 1024], f32, tag="acc",
                                 name=f"pvm{b}_{qt}")
                pts = {}

                def j0_of(ks):
                    m = ks - qt * 4
                    return m * P if m >= 1 else 0

                def emit_pv(kk):
                    jj = j0_of(kk)
                    ptk = pts.pop(kk)
                    for h in range(2):
                        nc.tensor.matmul(
                            pvm[:, h * 512 + jj:(h + 1) * 512],
                            v_sb[:, b * (S // P) + kk, h * P:(h + 1) * P],
                            ptk[:, h, jj:],
                            start=(kk == 0), stop=(kk == nks - 1))

                for ks in range(nks):
                    kcols = slice(b * S + ks * P, b * S + (ks + 1) * P)
                    m = ks - qt * 4
                    j0 = j0_of(ks)
                    qv = slice(b * S + qt * RB + j0, b * S + (qt + 1) * RB)
                    st = psum.tile([P, 1024], f32, tag="st",
                                   name=f"st{b}_{qt}_{ks}")
                    stv = st[:].rearrange("p (h c) -> p h c", h=2)
                    pt = ptp.tile([P, 2, RB], bf16, tag="pt")
                    pts[ks] = pt
                    for h in range(2):
                        hsl = slice(h * 64, (h + 1) * 64)
                        nc.tensor.matmul(st[:, h * 512 + j0:(h + 1) * 512],
                                         qkT[hsl, 1, kcols], qkT[hsl, 0, qv],
                                         start=True, stop=True)
                    nc.scalar.activation(pt[:, :, j0:], stv[:, :, j0:],
                                         mybir.ActivationFunctionType.Exp)
                    if m >= 0:
                        nc.vector.tensor_mul(pt[:, :, j0:j0 + P],
                                             pt[:, :, j0:j0 + P], tri_sb[:])
                    if fillers:
                        fillers.popleft()()
                    if ks >= 2:
                        emit_pv(ks - 2)
                for kk in (nks - 2, nks - 1):
                    emit_pv(kk)
                return (b, qt, qcols, pvm)

            # ---------- master schedule ----------
            dq = deque()          # deferred output-projection chunks
            for ch in a_chunks(0):
                ch()
            prev = None
            for rt in range(NRB):
                b, qt = rt // QT_PER_B, rt % QT_PER_B
                fillers = deque()
                if prev is not None:
                    fillers.extend(epi_chunks(prev))
                if rt < NRB - 1:
                    fillers.extend(a_chunks(rt + 1))
                if rt == 4:
                    dq.extend(d_chunk(rr) for rr in range(16))
                nks = qt * 4 + 4
                while len(fillers) < nks and dq:
                    fillers.append(dq.popleft())
                prev = phase_c(b, qt, fillers)
                while fillers:
                    fillers.popleft()()
            for ch in epi_chunks(prev):
                ch()
            while dq:
                dq.popleft()()
            for rr in range(16, KSUB):
                d_chunk(rr)()

    nc.finalize()
    return nc


def _host_inputs():
    t = np.arange(32, dtype=np.float64)
    inv_freq = 1.0 / (ROPE_BASE ** (2.0 * t / DH))
    pos = np.arange(S, dtype=np.float64)
    ang = pos[None, :] * inv_freq[:, None]          # [32, S]
    cos32 = np.cos(ang).astype(np.float32)
    sin32 = np.sin(ang).astype(np.float32)
    cc = np.tile(cos32, (4, 1))                     # [128, S]
    ss = np.concatenate([-sin32, sin32, -sin32, sin32], axis=0)  # [128, S]
    cc2 = np.concatenate([cc, cc], axis=1)          # [128, 2S] (q|k dup)
    ss2 = np.concatenate([ss, ss], axis=1)

    ii = np.arange(P)[:, None]
    uu = np.arange(P)[None, :]
    tri = (uu >= ii).astype(np.float32)             # [128, 128]
    tri2 = np.concatenate([tri, tri], axis=1)       # [128, 256]

    perm64 = np.concatenate([np.arange(0, 64, 2), np.arange(1, 64, 2)])
    return cc2, ss2, tri2, perm64


def _in_maps(x, Wq, bq, Wk, bk, Wv, bv, Wo):
    cc2, ss2, tri2, perm64 = _host_inputs()
    # swap32 permutation matrix: psw[m,:] = praw[src(m),:], src = xor-32
    # within each 64-block -> pswm[k, m] = 1 iff k == src(m)
    pswm = np.zeros((P, P), dtype=np.float32)
    for m_ in range(P):
        k_ = (m_ & ~63) | ((m_ + 32) & 63)
        pswm[k_, m_] = 1.0
    pswm = pswm.astype(nbf16)
    x2 = np.ascontiguousarray(x.reshape(NROWS, D))
    # xT block-major: xT[p, rt, d, c] = x[512*rt + c, 128*d + p]
    xT = np.ascontiguousarray(
        x2.reshape(NRB, RB, DSUB, P).transpose(3, 0, 2, 1)
        .reshape(P, NRB * DSUB * RB)).astype(nbf16)
    perm128 = np.concatenate([perm64, perm64 + 64])
    cc2b = cc2.astype(nbf16)
    ss2b = ss2.astype(nbf16)
    tri2b = tri2.astype(nbf16)
    maps = []
    for c in range(8):
        sl = slice(c * CH, (c + 1) * CH)
        maps.append({
            "xT": xT,
            "wqT": np.ascontiguousarray(
                (Wq[sl][perm128] * 0.125).T).astype(nbf16),
            "wkT": np.ascontiguousarray(Wk[sl][perm128].T).astype(nbf16),
            "wvT": np.ascontiguousarray(Wv[sl].T).astype(nbf16),
            "woT": np.ascontiguousarray(Wo[:, sl].T).astype(nbf16),
            "bq": (bq[sl][perm128] * 0.125).reshape(CH, 1).copy(),
            "bk": bk[sl][perm128].reshape(CH, 1).copy(),
            "bv": bv[sl].reshape(CH, 1).copy(),
            "cc2": cc2b, "ss2": ss2b, "tri": tri2b, "pswm": pswm,
        })
    return maps


def kernel(x, Wq, bq, Wk, bk, Wv, bv, Wo, bo):
    x = np.asarray(x, dtype=np.float32)
    Wq = np.asarray(Wq, dtype=np.float32)
    Wk = np.asarray(Wk, dtype=np.float32)
    Wv = np.asarray(Wv, dtype=np.float32)
    Wo = np.asarray(Wo, dtype=np.float32)
    bq = np.asarray(bq, dtype=np.float32)
    bk = np.asarray(bk, dtype=np.float32)
    bv = np.asarray(bv, dtype=np.float32)
    bo = np.asarray(bo, dtype=np.float32)

    if "nc" not in _CACHE:
        _CACHE["nc"] = _build()
    nc = _CACHE["nc"]

    res = run_bass_kernel_spmd(nc, _in_maps(x, Wq, bq, Wk, bk, Wv, bv, Wo),
                               core_ids=list(range(8)))
    out = np.zeros((NROWS, D), dtype=np.float32)
    for r in res.results:
        out += r["out"].astype(np.float32)
    out += bo[None, :]
    return out.reshape(B, S, D)
